# revision 7
# baseline (speedup 1.0000x reference)
"""Trainium2 Bass kernel for MACE-style message-passing convolution.

Reference computation (per edge e with sender s, receiver r):
    msg0 = node_feats[s]                          # [64] scalars
    u    = vectors[e] / |vectors[e]|
    Y1   = sqrt(3) u ;  Y2 = 5 quadratic harmonics of u
    mix  = MLP(radial[e])                         # [192] = m0|m1|m2
    msg  = [msg0*m0, (msg0 (x) Y1)*m1, (msg0 (x) Y2)*m2]   # [576]
    out[r] += msg / sqrt(16)

Strategy (8 NeuronCores, SPMD):
  * Host: sort edges by receiver, shard NODES across cores (core c owns
    nodes [2048c, 2048(c+1))) so each core gets a contiguous slice of
    sorted edges -> no collective needed.
  * Host bakes data layouts only (gather of node_feats by sender, edge
    window layout, one-hot scatter matrices, weight folding). All FLOPs
    of the reference run on-device.
  * Device per core, tuned against the HW perfetto trace:
      - MLP on PE in fp32r (full rate at 512-wide moving dim), silu on ACT.
      - mix matmul in bf16 (fp32r at 256-wide runs 1/4 rate on silicon).
      - ab = mix*msg0 on DVE (all-bf16 SBUF operands -> 2x mode).
      - tensor products (broadcast ops, no 2x possible) split between
        DVE and the otherwise-idle GPSIMD engine.
      - segment-sum via one-hot bf16 matmuls accumulating in a single
        [512|64] PSUM tile per 128-node block; one ACT evacuation
        instruction per block + 2 output DMAs.
      - chunk pipeline software-pipelined 2 deep.
"""

import os
import sys
from contextlib import ExitStack

import numpy as np

sys.path.insert(0, "/opt/trn_rl_repo")

import ml_dtypes  # noqa: E402

import concourse.bass as bass  # noqa: E402
import concourse.bacc as bacc  # noqa: E402
import concourse.tile as tile  # noqa: E402
from concourse import mybir  # noqa: E402

N_CORES = 8
N_NODES = 16384
N_EDGES = 262144
MUL = 64
N_BASIS = 8
HIDDEN = 64
NUM_IRREPS = 3 * MUL  # 192
MSG_W = 9 * MUL  # 576
NODES_PER_CORE = N_NODES // N_CORES  # 2048
BLOCKS = NODES_PER_CORE // 128  # 16 node-blocks of 128
WIN = 128  # edges per window (matmul K)
CHUNK_E = 1024  # edges per MLP chunk (2 stacked groups of 512)

# Tensor products: GPSIMD owns windows [0,4) (fed by the g=0 half-chunk),
# DVE owns windows [4,8) (fed by g=1). Separate msg/ab tiles per half so
# the two engines never touch the same tile (the tile framework serializes
# cross-engine writers of one tile). GPSIMD handles full-width contiguous
# runs at ~1.8ns/col but chokes on column-sliced APs.
POOL_W = 4

F32 = mybir.dt.float32
BF16 = mybir.dt.bfloat16
F32R = mybir.dt.float32r
AF = mybir.ActivationFunctionType
OP = mybir.AluOpType


def _silu_norm():
    x = np.linspace(-12.0, 12.0, 24001)
    p = np.exp(-0.5 * x * x) / np.sqrt(2.0 * np.pi)
    s = x / (1.0 + np.exp(-x))
    trapz = getattr(np, "trapz", None) or np.trapezoid
    return float(1.0 / np.sqrt(trapz(s * s * p, x)))


def _prep(vectors, node_feats, radial_embedding, W0, W1, W2, W3,
          senders, receivers):
    """Host-side data marshaling: sort/shard/pad/bake layouts."""
    snd = np.asarray(senders).astype(np.int64)
    rcv = np.asarray(receivers).astype(np.int64)
    vectors = np.asarray(vectors, dtype=np.float32)
    node_feats = np.asarray(node_feats, dtype=np.float32)
    radial = np.asarray(radial_embedding, dtype=np.float32)

    perm = np.argsort(rcv, kind="stable")
    rcv_s = rcv[perm]
    snd_s = snd[perm]
    v_s = vectors[perm]
    rad_s = radial[perm]

    bounds = np.searchsorted(rcv_s, np.arange(N_CORES + 1) * NODES_PER_CORE)
    e_counts = np.diff(bounds)
    E_pad = int(np.ceil(e_counts.max() / CHUNK_E) * CHUNK_E)
    W = E_pad // WIN  # windows per core
    CH = E_pad // CHUNK_E

    sn = _silu_norm()
    W0e = (np.asarray(W0, np.float32) / np.sqrt(N_BASIS))
    W1e = (np.asarray(W1, np.float32) * sn / np.sqrt(HIDDEN))
    W2e = (np.asarray(W2, np.float32) * sn / np.sqrt(HIDDEN))
    W3e = (np.asarray(W3, np.float32) * sn / np.sqrt(HIDDEN) / 4.0).copy()
    W3e[:, MUL:2 * MUL] *= np.sqrt(3.0)  # fold Y1 = sqrt(3) u

    def blockdiag(w):
        k, m = w.shape
        out = np.zeros((2 * k, 2 * m), np.float32)
        out[:k, :m] = w
        out[k:, m:] = w
        return out

    w01 = blockdiag(W0e)
    w1b = blockdiag(W1e)
    w2b = blockdiag(W2e)
    w3e = W3e.astype(ml_dtypes.bfloat16)  # [64, 192] bf16

    # Per-core block->window ranges, unified across cores (SPMD: one program)
    core = {}
    blo_all = np.full((N_CORES, BLOCKS), 10**9, np.int64)
    bhi_all = np.full((N_CORES, BLOCKS), -1, np.int64)
    for c in range(N_CORES):
        lo, hi = bounds[c], bounds[c + 1]
        ec = hi - lo
        rl = rcv_s[lo:hi] - c * NODES_PER_CORE  # local node ids [0, 2048)
        rl_pad = np.full(E_pad, -1, np.int64)
        rl_pad[:ec] = rl
        # block edge ranges within this core's (padded) edge list
        bb = np.searchsorted(rl, np.arange(BLOCKS + 1) * 128)
        for b in range(BLOCKS):
            if bb[b + 1] > bb[b]:
                blo_all[c, b] = bb[b] // WIN
                bhi_all[c, b] = (bb[b + 1] - 1) // WIN
        core[c] = dict(lo=lo, hi=hi, ec=ec, rl_pad=rl_pad)
    B_LO = blo_all.min(axis=0)
    B_HI = bhi_all.max(axis=0)
    for b in range(BLOCKS):
        if B_HI[b] < B_LO[b]:
            B_LO[b], B_HI[b] = 0, -1  # empty everywhere -> memset path
    # pair list in window-major emission order
    pairs = []  # (w, b, start, stop)
    for w in range(W):
        for b in range(BLOCKS):
            if B_LO[b] <= w <= B_HI[b]:
                pairs.append((w, b, w == B_LO[b], w == B_HI[b]))
    n_pairs = len(pairs)

    in_maps = []
    for c in range(N_CORES):
        cc = core[c]
        lo, ec = cc["lo"], cc["ec"]
        # padded per-core edge arrays
        v_pad = np.zeros((E_pad, 3), np.float32)
        v_pad[:, 0] = 1.0
        v_pad[:ec] = v_s[lo:lo + ec]
        rad_pad = np.zeros((E_pad, N_BASIS), np.float32)
        rad_pad[:ec] = rad_s[lo:lo + ec]
        snd_pad = np.zeros(E_pad, np.int64)
        snd_pad[:ec] = snd_s[lo:lo + ec]

        msg0 = node_feats[snd_pad]  # [E_pad, 64] host gather (layout only)
        msg0 = (msg0.reshape(W, WIN, MUL).transpose(1, 0, 2)
                .reshape(128, W * MUL).astype(ml_dtypes.bfloat16))

        vint = v_pad.reshape(W, WIN, 3).transpose(1, 0, 2).reshape(128, 3 * W)

        r4 = rad_pad.reshape(CH, 2, 512, N_BASIS)
        rad16 = np.ascontiguousarray(
            r4.transpose(1, 3, 0, 2).reshape(16, CH * 512))

        # one-hot scatter matrices per (w, b) pair, bf16 (exact 0/1)
        rlp = cc["rl_pad"]
        ohs = np.zeros((n_pairs, WIN, 128), ml_dtypes.bfloat16)
        ar = np.arange(128)
        for i, (w, b, _, _) in enumerate(pairs):
            rloc = rlp[w * WIN:(w + 1) * WIN] - 128 * b
            ohs[i] = (rloc[:, None] == ar[None, :]).astype(ml_dtypes.bfloat16)
        ohs = ohs.transpose(1, 0, 2).reshape(WIN, n_pairs * 128)

        in_maps.append({
            "msg0": np.ascontiguousarray(msg0),
            "vint": np.ascontiguousarray(vint),
            "rad16": np.ascontiguousarray(rad16),
            "ohs": np.ascontiguousarray(ohs),
            "w01": w01, "w1b": w1b, "w2b": w2b, "w3e": w3e,
        })

    meta = dict(W=W, CH=CH, pairs=pairs, n_pairs=n_pairs,
                B_LO=B_LO, B_HI=B_HI)
    return in_maps, meta


def _build(meta, sim_safe=False):
    """Build the SPMD Bass/Tile program (identical across cores).

    sim_safe: CoreSim doesn't implement the Silu ACT function; emit
    Sigmoid + elementwise multiply instead (identical math) for sim runs.
    """
    W = meta["W"]
    CH = meta["CH"]
    pairs = meta["pairs"]
    n_pairs = meta["n_pairs"]

    FR = F32 if sim_safe else F32R
    nc = bacc.Bacc("TRN2", target_bir_lowering=False, debug=False)
    msg0_d = nc.declare_dram_parameter("msg0", [128, W * MUL], BF16, isOutput=False)
    vint_d = nc.declare_dram_parameter("vint", [128, 3 * W], F32, isOutput=False)
    rad_d = nc.declare_dram_parameter("rad16", [16, CH * 512], FR, isOutput=False)
    ohs_d = nc.declare_dram_parameter("ohs", [128, n_pairs * 128], BF16, isOutput=False)
    w01_d = nc.declare_dram_parameter("w01", [16, 128], FR, isOutput=False)
    w1b_d = nc.declare_dram_parameter("w1b", [128, 128], FR, isOutput=False)
    w2b_d = nc.declare_dram_parameter("w2b", [128, 128], FR, isOutput=False)
    w3e_d = nc.declare_dram_parameter("w3e", [64, NUM_IRREPS], BF16, isOutput=False)
    out_d = nc.declare_dram_parameter("out", [NODES_PER_CORE, MSG_W], F32,
                                      isOutput=True)

    C15 = float(np.sqrt(15.0))
    C5H = float(np.sqrt(5.0) / 2.0)

    def silu(out_ap, in_ap):
        if sim_safe:
            nc.scalar.activation(out_ap, in_ap, AF.Sigmoid)
            nc.vector.tensor_tensor(out_ap, out_ap, in_ap, OP.mult)
        else:
            nc.scalar.activation(out_ap, in_ap, AF.Silu)

    assert POOL_W == 4  # mix/ab half-chunks are fixed 4-window groups
    # sph strips: ~CH/4 chunks each, chunk-aligned
    NSTRIP = min(4, CH)
    base_sz, rem = divmod(CH, NSTRIP)
    strip_chunks = []  # (chunk_lo, chunk_hi)
    c0 = 0
    for q in range(NSTRIP):
        sz = base_sz + (1 if q < rem else 0)
        strip_chunks.append((c0, c0 + sz))
        c0 += sz
    strip_of_chunk = {}
    for q, (lo, hi) in enumerate(strip_chunks):
        for j in range(lo, hi):
            strip_of_chunk[j] = q

    with tile.TileContext(nc) as tc, ExitStack() as ctx:
        const = ctx.enter_context(tc.tile_pool(name="const", bufs=1))
        sphp = ctx.enter_context(tc.tile_pool(name="sph", bufs=2))
        y8p = ctx.enter_context(tc.tile_pool(name="y8p", bufs=1))
        radp = ctx.enter_context(tc.tile_pool(name="rad", bufs=2))
        hp = ctx.enter_context(tc.tile_pool(name="hp", bufs=2, space="PSUM"))
        hact = ctx.enter_context(tc.tile_pool(name="hact", bufs=2))
        h3p = ctx.enter_context(tc.tile_pool(name="h3", bufs=2))
        mixp = ctx.enter_context(tc.tile_pool(name="mixp", bufs=1, space="PSUM"))
        mxs = ctx.enter_context(tc.tile_pool(name="mxs", bufs=2))
        abp = ctx.enter_context(tc.tile_pool(name="ab", bufs=8))
        m0p = ctx.enter_context(tc.tile_pool(name="m0", bufs=2))
        msgp = ctx.enter_context(tc.tile_pool(name="msg", bufs=8))
        ohp = ctx.enter_context(tc.tile_pool(name="oh", bufs=3))
        aggp = ctx.enter_context(tc.tile_pool(name="agg", bufs=2, space="PSUM"))
        outp = ctx.enter_context(tc.tile_pool(name="outs", bufs=2))

        LOOKAHEAD = 2
        rad_tiles = {}
        m0_tiles = {}

        def prefetch_chunk(j):
            radt = radp.tile([16, 512], FR, tag="radt")
            nc.sync.dma_start(radt[:], rad_d[:, j * 512:(j + 1) * 512])
            rad_tiles[j] = radt
            m0t = m0p.tile([128, 8 * MUL], BF16, tag="m0t")
            nc.sync.dma_start(
                m0t[:], msg0_d[:, j * 8 * MUL:(j + 1) * 8 * MUL])
            m0_tiles[j] = m0t

        # chunk 0 inputs + weights first so PE can start immediately
        prefetch_chunk(0)
        w01t = const.tile([16, 128], FR)
        nc.sync.dma_start(w01t[:], w01_d[:])
        w1bt = const.tile([128, 128], FR)
        nc.sync.dma_start(w1bt[:], w1b_d[:])
        w2bt = const.tile([128, 128], FR)
        nc.sync.dma_start(w2bt[:], w2b_d[:])
        # two copies of W3 (partitions 0:64 and 64:128) so the mix matmul's
        # lhsT (h3 slice) and rhs share a base partition
        w3et = const.tile([128, NUM_IRREPS], BF16)
        nc.sync.dma_start(w3et[0:64, :], w3e_d[:])
        nc.sync.dma_start(w3et[64:128, :], w3e_d[:])

        # y8[q] [128, Wq, 8] bf16: per window cols [u_x u_y u_z | y2_0..y2_4]
        y8_tiles = {}
        vt_tiles = {}

        def prefetch_vt(q):
            lo, hi = strip_chunks[q]
            Wq = (hi - lo) * 8
            vt = sphp.tile([128, 3 * Wq], F32, tag="vt", name=f"vt{q}")
            nc.sync.dma_start(vt[:], vint_d[:, lo * 24:lo * 24 + 3 * Wq])
            vt_tiles[q] = vt

        def sph_strip(q):
            lo, hi = strip_chunks[q]
            Wq = (hi - lo) * 8
            vt = vt_tiles[q]
            vsq = sphp.tile([128, 3 * Wq], F32, tag="vsq")
            nc.vector.tensor_tensor(vsq[:], vt[:], vt[:], OP.mult)
            s2 = sphp.tile([128, Wq], F32, tag="s2")
            nc.vector.tensor_reduce(
                s2[:], vsq[:].rearrange("p (w k) -> p w k", k=3),
                mybir.AxisListType.X, OP.add)
            rs = sphp.tile([128, Wq], F32, tag="rs")
            nc.vector.reciprocal(rs[:], s2[:])
            rinv = sphp.tile([128, Wq], F32, tag="rinv")  # 1/|v|
            nc.scalar.activation(rinv[:], rs[:], AF.Sqrt)
            u3 = sphp.tile([128, 3 * Wq], F32, tag="u3")
            nc.vector.tensor_tensor(
                u3[:].rearrange("p (w k) -> p w k", k=3),
                vt[:].rearrange("p (w k) -> p w k", k=3),
                rinv[:].unsqueeze(2).broadcast_to([128, Wq, 3]),
                OP.mult)
            ux = u3[:].rearrange("p (w k) -> p k w", k=3)[:, 0]
            uy = u3[:].rearrange("p (w k) -> p k w", k=3)[:, 1]
            uz = u3[:].rearrange("p (w k) -> p k w", k=3)[:, 2]
            y5 = sphp.tile([128, 5 * Wq], F32, tag="y5")
            y5v = y5[:].rearrange("p (w k) -> p k w", k=5)
            nc.vector.scalar_tensor_tensor(y5v[:, 0], ux, C15, uy,
                                           OP.mult, OP.mult)
            nc.vector.scalar_tensor_tensor(y5v[:, 1], uy, C15, uz,
                                           OP.mult, OP.mult)
            nc.vector.scalar_tensor_tensor(y5v[:, 2], uz, 3.0 * C5H, uz,
                                           OP.mult, OP.mult)
            nc.vector.tensor_scalar_add(y5v[:, 2], y5v[:, 2], -C5H)
            nc.vector.scalar_tensor_tensor(y5v[:, 3], ux, C15, uz,
                                           OP.mult, OP.mult)
            tpq = sphp.tile([128, 2 * Wq], F32, tag="tpq")
            nc.vector.tensor_tensor(tpq[:, :Wq], ux, uy, OP.add)
            nc.vector.tensor_tensor(tpq[:, Wq:], ux, uy, OP.subtract)
            nc.vector.scalar_tensor_tensor(y5v[:, 4], tpq[:, :Wq], C15 / 2.0,
                                           tpq[:, Wq:], OP.mult, OP.mult)
            y8 = y8p.tile([128, Wq * 8], BF16, name=f"y8_{q}")
            y8_tiles[q] = y8
            y8v = y8[:].rearrange("p (w k) -> p w k", k=8)
            nc.vector.tensor_copy(
                y8v[:, :, 0:3], u3[:].rearrange("p (w k) -> p w k", k=3))
            nc.vector.tensor_copy(
                y8v[:, :, 3:8],
                y5[:].rearrange("p (w k) -> p w k", k=5))

        prefetch_vt(0)

        # segment bookkeeping: pairs grouped by msg chunk, split in 4 parts
        chunk_pair_rng = {}
        for i, (w, b, _, _) in enumerate(pairs):
            jj = w // 8
            lo, hi = chunk_pair_rng.get(jj, (i, i))
            chunk_pair_rng[jj] = (min(lo, i), i + 1)

        pair_i = 0
        agg_t = {}
        ab_tiles = {}
        msg_tiles = {}
        oh_tiles = {}

        def prefetch_oh(gi):
            if gi in oh_tiles or gi * 8 >= n_pairs:
                return
            oht = ohp.tile([128, 8 * 128], BF16, tag="oh", bufs=3)
            n_in = min(8 * 128, (n_pairs - gi * 8) * 128)
            nc.sync.dma_start(
                oht[:, :n_in], ohs_d[:, gi * 8 * 128:gi * 8 * 128 + n_in])
            oh_tiles[gi] = oht

        def seg_part(jj, part):
            nonlocal pair_i
            if jj < 0 or jj not in chunk_pair_rng:
                return
            lo, hi = chunk_pair_rng[jj]
            n = hi - lo
            stop_i = hi if part == 3 else lo + ((part + 1) * n) // 4
            mP, mD = msg_tiles[jj]
            msgvP = mP[:].rearrange("p (w f) -> p w f", f=512)
            msgvD = mD[:].rearrange("p (w f) -> p w f", f=512)
            aP, aD = ab_tiles[jj]
            abvP = aP[:].rearrange("p (w l c) -> p w l c", l=3, c=MUL)
            abvD = aD[:].rearrange("p (w l c) -> p w l c", l=3, c=MUL)
            while pair_i < stop_i:
                w, b, is_start, is_stop = pairs[pair_i]
                wj = w % 8
                gi, gs = divmod(pair_i, 8)
                prefetch_oh(gi)
                prefetch_oh(gi + 1)
                if is_start:
                    # single PSUM tile: [0:512] = l1|l2, [512:576] = l0
                    agg_t[b] = aggp.tile([128, 640], F32, tag="agg",
                                         name=f"agg{b}")
                at = agg_t[b]
                lhs = oh_tiles[gi][:, gs * 128:(gs + 1) * 128]
                rhs_b = (msgvP[:, wj, :] if wj < POOL_W
                         else msgvD[:, wj - POOL_W, :])
                rhs_a = (abvP[:, wj, 0, :] if wj < POOL_W
                         else abvD[:, wj - POOL_W, 0, :])
                nc.tensor.matmul(at[:, 0:512], lhs, rhs_b,
                                 start=is_start, stop=is_stop)
                nc.tensor.matmul(at[:, 512:576], lhs, rhs_a,
                                 start=is_start, stop=is_stop)
                if is_stop:
                    ot = outp.tile([128, MSG_W], F32, tag="ot")
                    nc.scalar.activation(ot[:], at[:, 0:576], AF.Copy)
                    nc.sync.dma_start(
                        out_d[b * 128:(b + 1) * 128, 0:MUL],
                        ot[:, 512:576])
                    nc.sync.dma_start(
                        out_d[b * 128:(b + 1) * 128, MUL:MSG_W],
                        ot[:, 0:512])
                pair_i += 1

        def mlp_stage(j, layer):
            # layer 0: h1 matmul; 1: silu1+h2; 2: silu2+h3; 3: silu3
            if layer == 0:
                h1ps = hp.tile([128, 512], F32, tag="hps", name=f"h1ps{j}")
                nc.tensor.matmul(h1ps[:], w01t[:], rad_tiles[j][:])
                return h1ps
            prev = mlp_state[j]
            if layer == 1:
                h1 = hact.tile([128, 512], FR, tag="h12")
                silu(h1[:], prev[:])
                h2ps = hp.tile([128, 512], F32, tag="hps", name=f"h2ps{j}")
                nc.tensor.matmul(h2ps[:], w1bt[:], h1[:])
                return h2ps
            if layer == 2:
                h2 = hact.tile([128, 512], FR, tag="h12")
                silu(h2[:], prev[:])
                h3ps = hp.tile([128, 512], F32, tag="hps", name=f"h3ps{j}")
                nc.tensor.matmul(h3ps[:], w2bt[:], h2[:])
                return h3ps
            h3 = h3p.tile([128, 512], BF16)
            silu(h3[:], prev[:])
            return h3

        def chunk_tail(j, h3):
            # mix + ab + tensor products; GPSIMD owns windows 0:4 (g=0),
            # DVE owns windows 4:8 (g=1), with disjoint msg/ab tiles.
            q = strip_of_chunk[j]
            wbase = (j - strip_chunks[q][0]) * 8
            y8v = y8_tiles[q][:].rearrange("p (w k) -> p w k", k=8)
            m0t = m0_tiles[j]
            mP = msgp.tile([128, POOL_W * 512], BF16, tag="msgP",
                           bufs=LOOKAHEAD + 2)
            mD = msgp.tile([128, (8 - POOL_W) * 512], BF16, tag="msgD",
                           bufs=LOOKAHEAD + 2)
            msg_tiles[j] = (mP, mD)
            aP = abp.tile([128, POOL_W * NUM_IRREPS], BF16, tag="abP",
                          name=f"abP_{j}", bufs=LOOKAHEAD + 2)
            aD = abp.tile([128, (8 - POOL_W) * NUM_IRREPS], BF16, tag="abD",
                          name=f"abD_{j}", bufs=LOOKAHEAD + 2)
            ab_tiles[j] = (aP, aD)
            for g in range(2):  # half-chunks of 4 windows
                nw = POOL_W if g == 0 else 8 - POOL_W
                abt = aP if g == 0 else aD
                abv = abt[:].rearrange("p (w l c) -> p w l c", l=3, c=MUL)
                # mix: edge-major via stationary-h3 matmuls, bf16 full rate;
                # 256-col PSUM slots so each output stays inside one bank
                mixt = mixp.tile([128, 4 * 256], F32, tag="mixt")
                for t4 in range(4):
                    t = g * 4 + t4
                    half, coff = ((0, t * 128) if t < 4
                                  else (64, (t - 4) * 128))
                    nc.tensor.matmul(
                        mixt[:, t4 * 256:t4 * 256 + NUM_IRREPS],
                        h3[half:half + 64, coff:coff + 128],
                        w3et[half:half + 64, :])
                # evacuate mix PSUM -> bf16 SBUF (ACT)
                mixs = mxs.tile([128, 4 * NUM_IRREPS], BF16, tag="mixs")
                nc.scalar.activation(
                    mixs[:].rearrange("p (w x) -> p w x", x=192),
                    mixt[:].rearrange("p (w x) -> p w x", x=256)
                    [:, :, 0:192], AF.Copy)
                # A = mix (*) msg0 broadcast over l  (all-bf16 SBUF, 2x)
                nc.vector.tensor_tensor(
                    abv,
                    mixs[:].rearrange("p (w l c) -> p w l c", l=3, c=MUL),
                    m0t[:].rearrange("p (w c) -> p w c", c=MUL)
                        [:, g * 4:g * 4 + nw]
                        .unsqueeze(2).broadcast_to([128, nw, 3, MUL]),
                    OP.mult)
                # tensor products for this half (broadcast ops, no 2x)
                eng = nc.gpsimd if g == 0 else nc.vector
                mt = mP if g == 0 else mD
                msgv = mt[:].rearrange("p (w f) -> p w f", f=512)
                y8c = y8v[:, wbase + g * 4:wbase + g * 4 + nw]
                eng.tensor_tensor(
                    msgv[:, :, 0:3 * MUL].rearrange(
                        "p w (c k) -> p w c k", k=3),
                    y8c[:, :, 0:3].unsqueeze(2)
                        .broadcast_to([128, nw, MUL, 3]),
                    abv[:, :, 1, :].unsqueeze(3)
                        .broadcast_to([128, nw, MUL, 3]),
                    OP.mult)
                eng.tensor_tensor(
                    msgv[:, :, 3 * MUL:8 * MUL].rearrange(
                        "p w (c k) -> p w c k", k=5),
                    y8c[:, :, 3:8].unsqueeze(2)
                        .broadcast_to([128, nw, MUL, 5]),
                    abv[:, :, 2, :].unsqueeze(3)
                        .broadcast_to([128, nw, MUL, 5]),
                    OP.mult)

        mlp_state = {}
        for j in range(CH + LOOKAHEAD):
            jj = j - LOOKAHEAD
            live = j < CH
            if live:
                if j + 1 < CH:
                    prefetch_chunk(j + 1)
                q = strip_of_chunk[j]
                if j == strip_chunks[q][0]:
                    if q + 1 < NSTRIP:
                        prefetch_vt(q + 1)
                    sph_strip(q)
                mlp_state[j] = mlp_stage(j, 0)
            seg_part(jj, 0)
            if live:
                mlp_state[j] = mlp_stage(j, 1)
            seg_part(jj, 1)
            if live:
                mlp_state[j] = mlp_stage(j, 2)
            seg_part(jj, 2)
            if live:
                h3 = mlp_stage(j, 3)
                chunk_tail(j, h3)
            seg_part(jj, 3)
        # empty blocks (defensive): write zeros
        empty = [b for b in range(BLOCKS) if meta["B_HI"][b] < meta["B_LO"][b]]
        if empty:
            zt = const.tile([128, MSG_W], F32)
            nc.vector.memset(zt[:], 0.0)
            for b in empty:
                nc.sync.dma_start(out_d[b * 128:(b + 1) * 128, :], zt[:])
    nc.compile()
    return nc


def kernel(**inputs) -> np.ndarray:
    in_maps, meta = _prep(**inputs)
    nc = _build(meta)
    from concourse.bass_utils import run_bass_kernel_spmd
    res = run_bass_kernel_spmd(nc, in_maps, list(range(N_CORES)))
    outs = [np.asarray(res.results[c]["out"], np.float32)
            for c in range(N_CORES)]
    return np.concatenate(outs, axis=0)


if __name__ == "__main__":
    import reference
    ins = {k: np.asarray(v) for k, v in reference.setup_inputs().items()}
    out = kernel(**ins)
    exp = np.asarray(reference.reference(**reference.setup_inputs()))
    err = np.abs(out - exp).max() / np.abs(exp).max()
    print("rel err:", err)


# revision 8
# speedup vs baseline: 1.1531x; 1.1531x over previous
"""Trainium2 Bass kernel for MACE-style message-passing convolution.

Reference computation (per edge e with sender s, receiver r):
    msg0 = node_feats[s]                          # [64] scalars
    u    = vectors[e] / |vectors[e]|
    Y1   = sqrt(3) u ;  Y2 = 5 quadratic harmonics of u
    mix  = MLP(radial[e])                         # [192] = m0|m1|m2
    msg  = [msg0*m0, (msg0 (x) Y1)*m1, (msg0 (x) Y2)*m2]   # [576]
    out[r] += msg / sqrt(16)

Strategy (8 NeuronCores, SPMD):
  * Host: sort edges by receiver, shard NODES across cores (core c owns
    nodes [2048c, 2048(c+1))) so each core gets a contiguous slice of
    sorted edges -> no collective needed.
  * Host bakes data layouts only (gather of node_feats by sender, edge
    window layout, one-hot scatter matrices, weight folding). All FLOPs
    of the reference run on-device.
  * Device per core, tuned against the HW perfetto trace:
      - MLP on PE in fp32r (full rate at 512-wide moving dim), silu on ACT.
      - mix matmul in bf16 (fp32r at 256-wide runs 1/4 rate on silicon).
      - ab = mix*msg0 on DVE (all-bf16 SBUF operands -> 2x mode).
      - tensor products (broadcast ops, no 2x possible) split between
        DVE and the otherwise-idle GPSIMD engine.
      - segment-sum via one-hot bf16 matmuls accumulating in a single
        [512|64] PSUM tile per 128-node block; one ACT evacuation
        instruction per block + 2 output DMAs.
      - chunk pipeline software-pipelined 2 deep.
"""

import os
import sys
from contextlib import ExitStack

import numpy as np

sys.path.insert(0, "/opt/trn_rl_repo")

import ml_dtypes  # noqa: E402

import concourse.bass as bass  # noqa: E402
import concourse.bacc as bacc  # noqa: E402
import concourse.tile as tile  # noqa: E402
from concourse import mybir  # noqa: E402

N_CORES = 8
N_NODES = 16384
N_EDGES = 262144
MUL = 64
N_BASIS = 8
HIDDEN = 64
NUM_IRREPS = 3 * MUL  # 192
MSG_W = 9 * MUL  # 576
NODES_PER_CORE = N_NODES // N_CORES  # 2048
BLOCKS = NODES_PER_CORE // 128  # 16 node-blocks of 128
WIN = 128  # edges per window (matmul K)
CHUNK_E = 1024  # edges per MLP chunk (2 stacked groups of 512)

# Tensor products: GPSIMD owns windows [0,4) (fed by the g=0 half-chunk),
# DVE owns windows [4,8) (fed by g=1). Separate msg/ab tiles per half so
# the two engines never touch the same tile (the tile framework serializes
# cross-engine writers of one tile). GPSIMD handles full-width contiguous
# runs at ~1.8ns/col but chokes on column-sliced APs.
POOL_W = 4

F32 = mybir.dt.float32
BF16 = mybir.dt.bfloat16
F32R = mybir.dt.float32r
AF = mybir.ActivationFunctionType
OP = mybir.AluOpType


def _silu_norm():
    x = np.linspace(-12.0, 12.0, 24001)
    p = np.exp(-0.5 * x * x) / np.sqrt(2.0 * np.pi)
    s = x / (1.0 + np.exp(-x))
    trapz = getattr(np, "trapz", None) or np.trapezoid
    return float(1.0 / np.sqrt(trapz(s * s * p, x)))


def _prep(vectors, node_feats, radial_embedding, W0, W1, W2, W3,
          senders, receivers):
    """Host-side data marshaling: sort/shard/pad/bake layouts."""
    snd = np.asarray(senders).astype(np.int64)
    rcv = np.asarray(receivers).astype(np.int64)
    vectors = np.asarray(vectors, dtype=np.float32)
    node_feats = np.asarray(node_feats, dtype=np.float32)
    radial = np.asarray(radial_embedding, dtype=np.float32)

    perm = np.argsort(rcv, kind="stable")
    rcv_s = rcv[perm]
    snd_s = snd[perm]
    v_s = vectors[perm]
    rad_s = radial[perm]

    bounds = np.searchsorted(rcv_s, np.arange(N_CORES + 1) * NODES_PER_CORE)
    e_counts = np.diff(bounds)
    E_pad = int(np.ceil(e_counts.max() / CHUNK_E) * CHUNK_E)
    W = E_pad // WIN  # windows per core
    CH = E_pad // CHUNK_E

    sn = _silu_norm()
    W0e = (np.asarray(W0, np.float32) / np.sqrt(N_BASIS))
    W1e = (np.asarray(W1, np.float32) * sn / np.sqrt(HIDDEN))
    W2e = (np.asarray(W2, np.float32) * sn / np.sqrt(HIDDEN))
    W3e = (np.asarray(W3, np.float32) * sn / np.sqrt(HIDDEN) / 4.0).copy()
    W3e[:, MUL:2 * MUL] *= np.sqrt(3.0)  # fold Y1 = sqrt(3) u

    def blockdiag(w):
        k, m = w.shape
        out = np.zeros((2 * k, 2 * m), np.float32)
        out[:k, :m] = w
        out[k:, m:] = w
        return out

    w01 = blockdiag(W0e)
    w1b = blockdiag(W1e)
    w2b = blockdiag(W2e)
    w3e = W3e.astype(ml_dtypes.bfloat16)  # [64, 192] bf16

    # Per-core block->window ranges, unified across cores (SPMD: one program)
    core = {}
    blo_all = np.full((N_CORES, BLOCKS), 10**9, np.int64)
    bhi_all = np.full((N_CORES, BLOCKS), -1, np.int64)
    for c in range(N_CORES):
        lo, hi = bounds[c], bounds[c + 1]
        ec = hi - lo
        rl = rcv_s[lo:hi] - c * NODES_PER_CORE  # local node ids [0, 2048)
        rl_pad = np.full(E_pad, -1, np.int64)
        rl_pad[:ec] = rl
        # block edge ranges within this core's (padded) edge list
        bb = np.searchsorted(rl, np.arange(BLOCKS + 1) * 128)
        for b in range(BLOCKS):
            if bb[b + 1] > bb[b]:
                blo_all[c, b] = bb[b] // WIN
                bhi_all[c, b] = (bb[b + 1] - 1) // WIN
        core[c] = dict(lo=lo, hi=hi, ec=ec, rl_pad=rl_pad)
    B_LO = blo_all.min(axis=0)
    B_HI = bhi_all.max(axis=0)
    for b in range(BLOCKS):
        if B_HI[b] < B_LO[b]:
            B_LO[b], B_HI[b] = 0, -1  # empty everywhere -> memset path
    # pair list in window-major emission order
    pairs = []  # (w, b, start, stop)
    for w in range(W):
        for b in range(BLOCKS):
            if B_LO[b] <= w <= B_HI[b]:
                pairs.append((w, b, w == B_LO[b], w == B_HI[b]))
    n_pairs = len(pairs)

    in_maps = []
    for c in range(N_CORES):
        cc = core[c]
        lo, ec = cc["lo"], cc["ec"]
        # padded per-core edge arrays
        v_pad = np.zeros((E_pad, 3), np.float32)
        v_pad[:, 0] = 1.0
        v_pad[:ec] = v_s[lo:lo + ec]
        rad_pad = np.zeros((E_pad, N_BASIS), np.float32)
        rad_pad[:ec] = rad_s[lo:lo + ec]
        snd_pad = np.zeros(E_pad, np.int64)
        snd_pad[:ec] = snd_s[lo:lo + ec]

        msg0 = node_feats[snd_pad]  # [E_pad, 64] host gather (layout only)
        msg0 = (msg0.reshape(W, WIN, MUL).transpose(1, 0, 2)
                .reshape(128, W * MUL).astype(ml_dtypes.bfloat16))

        vint = v_pad.reshape(W, WIN, 3).transpose(1, 0, 2).reshape(128, 3 * W)

        r4 = rad_pad.reshape(CH, 2, 512, N_BASIS)
        rad16 = np.ascontiguousarray(
            r4.transpose(1, 3, 0, 2).reshape(16, CH * 512))

        # one-hot scatter matrices per (w, b) pair, bf16 (exact 0/1)
        rlp = cc["rl_pad"]
        ohs = np.zeros((n_pairs, WIN, 128), ml_dtypes.bfloat16)
        ar = np.arange(128)
        for i, (w, b, _, _) in enumerate(pairs):
            rloc = rlp[w * WIN:(w + 1) * WIN] - 128 * b
            ohs[i] = (rloc[:, None] == ar[None, :]).astype(ml_dtypes.bfloat16)
        ohs = ohs.transpose(1, 0, 2).reshape(WIN, n_pairs * 128)

        in_maps.append({
            "msg0": np.ascontiguousarray(msg0),
            "vint": np.ascontiguousarray(vint),
            "rad16": np.ascontiguousarray(rad16),
            "ohs": np.ascontiguousarray(ohs),
            "w01": w01, "w1b": w1b, "w2b": w2b, "w3e": w3e,
        })

    meta = dict(W=W, CH=CH, pairs=pairs, n_pairs=n_pairs,
                B_LO=B_LO, B_HI=B_HI)
    return in_maps, meta


def _build(meta, sim_safe=False):
    """Build the SPMD Bass/Tile program (identical across cores).

    sim_safe: CoreSim doesn't implement the Silu ACT function; emit
    Sigmoid + elementwise multiply instead (identical math) for sim runs.
    """
    W = meta["W"]
    CH = meta["CH"]
    pairs = meta["pairs"]
    n_pairs = meta["n_pairs"]

    FR = F32 if sim_safe else F32R
    nc = bacc.Bacc("TRN2", target_bir_lowering=False, debug=False)
    msg0_d = nc.declare_dram_parameter("msg0", [128, W * MUL], BF16, isOutput=False)
    vint_d = nc.declare_dram_parameter("vint", [128, 3 * W], F32, isOutput=False)
    rad_d = nc.declare_dram_parameter("rad16", [16, CH * 512], FR, isOutput=False)
    ohs_d = nc.declare_dram_parameter("ohs", [128, n_pairs * 128], BF16, isOutput=False)
    w01_d = nc.declare_dram_parameter("w01", [16, 128], FR, isOutput=False)
    w1b_d = nc.declare_dram_parameter("w1b", [128, 128], FR, isOutput=False)
    w2b_d = nc.declare_dram_parameter("w2b", [128, 128], FR, isOutput=False)
    w3e_d = nc.declare_dram_parameter("w3e", [64, NUM_IRREPS], BF16, isOutput=False)
    out_d = nc.declare_dram_parameter("out", [NODES_PER_CORE, MSG_W], F32,
                                      isOutput=True)

    C15 = float(np.sqrt(15.0))
    C5H = float(np.sqrt(5.0) / 2.0)

    def silu(out_ap, in_ap):
        if sim_safe:
            nc.scalar.activation(out_ap, in_ap, AF.Sigmoid)
            nc.vector.tensor_tensor(out_ap, out_ap, in_ap, OP.mult)
        else:
            nc.scalar.activation(out_ap, in_ap, AF.Silu)

    assert POOL_W == 4  # mix/ab half-chunks are fixed 4-window groups
    # sph strips: ~CH/4 chunks each, chunk-aligned
    NSTRIP = min(4, CH)
    base_sz, rem = divmod(CH, NSTRIP)
    strip_chunks = []  # (chunk_lo, chunk_hi)
    c0 = 0
    for q in range(NSTRIP):
        sz = base_sz + (1 if q < rem else 0)
        strip_chunks.append((c0, c0 + sz))
        c0 += sz
    strip_of_chunk = {}
    for q, (lo, hi) in enumerate(strip_chunks):
        for j in range(lo, hi):
            strip_of_chunk[j] = q

    with tile.TileContext(nc) as tc, ExitStack() as ctx:
        const = ctx.enter_context(tc.tile_pool(name="const", bufs=1))
        sphp = ctx.enter_context(tc.tile_pool(name="sph", bufs=2))
        y8p = ctx.enter_context(tc.tile_pool(name="y8p", bufs=1))
        radp = ctx.enter_context(tc.tile_pool(name="rad", bufs=2))
        hp = ctx.enter_context(tc.tile_pool(name="hp", bufs=2, space="PSUM"))
        hact = ctx.enter_context(tc.tile_pool(name="hact", bufs=2))
        h3p = ctx.enter_context(tc.tile_pool(name="h3", bufs=2))
        mixp = ctx.enter_context(tc.tile_pool(name="mixp", bufs=1, space="PSUM"))
        mxs = ctx.enter_context(tc.tile_pool(name="mxs", bufs=2))
        abp = ctx.enter_context(tc.tile_pool(name="ab", bufs=8))
        m0p = ctx.enter_context(tc.tile_pool(name="m0", bufs=2))
        msgp = ctx.enter_context(tc.tile_pool(name="msg", bufs=8))
        ohp = ctx.enter_context(tc.tile_pool(name="oh", bufs=3))
        aggp = ctx.enter_context(tc.tile_pool(name="agg", bufs=2, space="PSUM"))
        outp = ctx.enter_context(tc.tile_pool(name="outs", bufs=2))

        LOOKAHEAD = 2
        rad_tiles = {}
        m0_tiles = {}

        def prefetch_chunk(j):
            radt = radp.tile([16, 512], FR, tag="radt")
            nc.sync.dma_start(radt[:], rad_d[:, j * 512:(j + 1) * 512])
            rad_tiles[j] = radt
            m0t = m0p.tile([128, 8 * MUL], BF16, tag="m0t")
            nc.sync.dma_start(
                m0t[:], msg0_d[:, j * 8 * MUL:(j + 1) * 8 * MUL])
            m0_tiles[j] = m0t

        # chunk 0 inputs + weights first so PE can start immediately
        prefetch_chunk(0)
        w01t = const.tile([16, 128], FR)
        nc.sync.dma_start(w01t[:], w01_d[:])
        w1bt = const.tile([128, 128], FR)
        nc.sync.dma_start(w1bt[:], w1b_d[:])
        w2bt = const.tile([128, 128], FR)
        nc.sync.dma_start(w2bt[:], w2b_d[:])
        # two copies of W3 (partitions 0:64 and 64:128) so the mix matmul's
        # lhsT (h3 slice) and rhs share a base partition
        w3et = const.tile([128, NUM_IRREPS], BF16)
        nc.sync.dma_start(w3et[0:64, :], w3e_d[:])
        nc.sync.dma_start(w3et[64:128, :], w3e_d[:])

        # y8[q] [128, Wq, 8] bf16: per window cols [u_x u_y u_z | y2_0..y2_4]
        y8_tiles = {}
        vt_tiles = {}

        def prefetch_vt(q):
            lo, hi = strip_chunks[q]
            Wq = (hi - lo) * 8
            vt = sphp.tile([128, 3 * Wq], F32, tag="vt", name=f"vt{q}")
            nc.sync.dma_start(vt[:], vint_d[:, lo * 24:lo * 24 + 3 * Wq])
            vt_tiles[q] = vt

        def sph_strip(q):
            lo, hi = strip_chunks[q]
            Wq = (hi - lo) * 8
            vt = vt_tiles[q]
            vsq = sphp.tile([128, 3 * Wq], F32, tag="vsq")
            nc.vector.tensor_tensor(vsq[:], vt[:], vt[:], OP.mult)
            s2 = sphp.tile([128, Wq], F32, tag="s2")
            nc.vector.tensor_reduce(
                s2[:], vsq[:].rearrange("p (w k) -> p w k", k=3),
                mybir.AxisListType.X, OP.add)
            rs = sphp.tile([128, Wq], F32, tag="rs")
            nc.vector.reciprocal(rs[:], s2[:])
            rinv = sphp.tile([128, Wq], F32, tag="rinv")  # 1/|v|
            nc.scalar.activation(rinv[:], rs[:], AF.Sqrt)
            u3 = sphp.tile([128, 3 * Wq], F32, tag="u3")
            nc.vector.tensor_tensor(
                u3[:].rearrange("p (w k) -> p w k", k=3),
                vt[:].rearrange("p (w k) -> p w k", k=3),
                rinv[:].unsqueeze(2).broadcast_to([128, Wq, 3]),
                OP.mult)
            ux = u3[:].rearrange("p (w k) -> p k w", k=3)[:, 0]
            uy = u3[:].rearrange("p (w k) -> p k w", k=3)[:, 1]
            uz = u3[:].rearrange("p (w k) -> p k w", k=3)[:, 2]
            y5 = sphp.tile([128, 5 * Wq], F32, tag="y5")
            y5v = y5[:].rearrange("p (w k) -> p k w", k=5)
            nc.vector.scalar_tensor_tensor(y5v[:, 0], ux, C15, uy,
                                           OP.mult, OP.mult)
            nc.vector.scalar_tensor_tensor(y5v[:, 1], uy, C15, uz,
                                           OP.mult, OP.mult)
            nc.vector.scalar_tensor_tensor(y5v[:, 2], uz, 3.0 * C5H, uz,
                                           OP.mult, OP.mult)
            nc.vector.tensor_scalar_add(y5v[:, 2], y5v[:, 2], -C5H)
            nc.vector.scalar_tensor_tensor(y5v[:, 3], ux, C15, uz,
                                           OP.mult, OP.mult)
            tpq = sphp.tile([128, 2 * Wq], F32, tag="tpq")
            nc.vector.tensor_tensor(tpq[:, :Wq], ux, uy, OP.add)
            nc.vector.tensor_tensor(tpq[:, Wq:], ux, uy, OP.subtract)
            nc.vector.scalar_tensor_tensor(y5v[:, 4], tpq[:, :Wq], C15 / 2.0,
                                           tpq[:, Wq:], OP.mult, OP.mult)
            y8 = y8p.tile([128, Wq * 8], BF16, name=f"y8_{q}")
            y8_tiles[q] = y8
            y8v = y8[:].rearrange("p (w k) -> p w k", k=8)
            nc.vector.tensor_copy(
                y8v[:, :, 0:3], u3[:].rearrange("p (w k) -> p w k", k=3))
            nc.vector.tensor_copy(
                y8v[:, :, 3:8],
                y5[:].rearrange("p (w k) -> p w k", k=5))

        prefetch_vt(0)

        # segment bookkeeping: pairs grouped by msg chunk, split in 4 parts
        chunk_pair_rng = {}
        for i, (w, b, _, _) in enumerate(pairs):
            jj = w // 8
            lo, hi = chunk_pair_rng.get(jj, (i, i))
            chunk_pair_rng[jj] = (min(lo, i), i + 1)

        pair_i = 0
        agg_t = {}
        ab_tiles = {}
        msg_tiles = {}
        oh_tiles = {}

        def prefetch_oh(gi):
            if gi in oh_tiles or gi * 8 >= n_pairs:
                return
            oht = ohp.tile([128, 8 * 128], BF16, tag="oh", bufs=3)
            n_in = min(8 * 128, (n_pairs - gi * 8) * 128)
            nc.sync.dma_start(
                oht[:, :n_in], ohs_d[:, gi * 8 * 128:gi * 8 * 128 + n_in])
            oh_tiles[gi] = oht

        def seg_part(jj, part):
            nonlocal pair_i
            if jj < 0 or jj not in chunk_pair_rng:
                return
            lo, hi = chunk_pair_rng[jj]
            n = hi - lo
            stop_i = hi if part == 3 else lo + ((part + 1) * n) // 4
            mP, mD = msg_tiles[jj]
            msgvP = mP[:].rearrange("p (w f) -> p w f", f=512)
            msgvD = mD[:].rearrange("p (w f) -> p w f", f=512)
            aP, aD = ab_tiles[jj]
            abvP = aP[:].rearrange("p (w l c) -> p w l c", l=3, c=MUL)
            abvD = aD[:].rearrange("p (w l c) -> p w l c", l=3, c=MUL)
            while pair_i < stop_i:
                w, b, is_start, is_stop = pairs[pair_i]
                wj = w % 8
                gi, gs = divmod(pair_i, 8)
                prefetch_oh(gi)
                prefetch_oh(gi + 1)
                if is_start:
                    # single PSUM tile: [0:512] = l1|l2, [512:576] = l0
                    agg_t[b] = aggp.tile([128, 640], F32, tag="agg",
                                         name=f"agg{b}")
                at = agg_t[b]
                lhs = oh_tiles[gi][:, gs * 128:(gs + 1) * 128]
                rhs_b = (msgvP[:, wj, :] if wj < POOL_W
                         else msgvD[:, wj - POOL_W, :])
                rhs_a = (abvP[:, wj, 0, :] if wj < POOL_W
                         else abvD[:, wj - POOL_W, 0, :])
                nc.tensor.matmul(at[:, 0:512], lhs, rhs_b,
                                 start=is_start, stop=is_stop)
                nc.tensor.matmul(at[:, 512:576], lhs, rhs_a,
                                 start=is_start, stop=is_stop)
                if is_stop:
                    ot = outp.tile([128, MSG_W], F32, tag="ot")
                    nc.scalar.activation(ot[:], at[:, 0:576], AF.Copy)
                    nc.sync.dma_start(
                        out_d[b * 128:(b + 1) * 128, 0:MUL],
                        ot[:, 512:576])
                    nc.sync.dma_start(
                        out_d[b * 128:(b + 1) * 128, MUL:MSG_W],
                        ot[:, 0:512])
                pair_i += 1

        def chunk_body(j):
            # MLP chunk j: 1024 edges as 2 stacked groups of 512
            h1ps = hp.tile([128, 512], F32, tag="hps")
            nc.tensor.matmul(h1ps[:], w01t[:], rad_tiles[j][:])
            h1 = hact.tile([128, 512], FR, tag="h12")
            silu(h1[:], h1ps[:])
            h2ps = hp.tile([128, 512], F32, tag="hps")
            nc.tensor.matmul(h2ps[:], w1bt[:], h1[:])
            h2 = hact.tile([128, 512], FR, tag="h12")
            silu(h2[:], h2ps[:])
            h3ps = hp.tile([128, 512], F32, tag="hps")
            nc.tensor.matmul(h3ps[:], w2bt[:], h2[:])
            h3 = h3p.tile([128, 512], BF16)
            silu(h3[:], h3ps[:])
            # mix + ab + tensor products; GPSIMD owns windows 0:4 (g=0),
            # DVE owns windows 4:8 (g=1), with disjoint msg/ab tiles so the
            # engines run concurrently (one tile per writer engine).
            q = strip_of_chunk[j]
            wbase = (j - strip_chunks[q][0]) * 8
            y8v = y8_tiles[q][:].rearrange("p (w k) -> p w k", k=8)
            m0t = m0_tiles[j]
            mP = msgp.tile([128, POOL_W * 512], BF16, tag="msgP",
                           bufs=LOOKAHEAD + 2)
            mD = msgp.tile([128, (8 - POOL_W) * 512], BF16, tag="msgD",
                           bufs=LOOKAHEAD + 2)
            msg_tiles[j] = (mP, mD)
            aP = abp.tile([128, POOL_W * NUM_IRREPS], BF16, tag="abP",
                          name=f"abP_{j}", bufs=LOOKAHEAD + 2)
            aD = abp.tile([128, (8 - POOL_W) * NUM_IRREPS], BF16, tag="abD",
                          name=f"abD_{j}", bufs=LOOKAHEAD + 2)
            ab_tiles[j] = (aP, aD)
            for g in range(2):  # half-chunks of 4 windows
                nw = POOL_W if g == 0 else 8 - POOL_W
                abt = aP if g == 0 else aD
                abv = abt[:].rearrange("p (w l c) -> p w l c", l=3, c=MUL)
                # mix: edge-major via stationary-h3 matmuls, bf16 full rate;
                # 256-col PSUM slots so each output stays inside one bank
                mixt = mixp.tile([128, 4 * 256], F32, tag="mixt")
                for t4 in range(4):
                    t = g * 4 + t4
                    half, coff = ((0, t * 128) if t < 4
                                  else (64, (t - 4) * 128))
                    nc.tensor.matmul(
                        mixt[:, t4 * 256:t4 * 256 + NUM_IRREPS],
                        h3[half:half + 64, coff:coff + 128],
                        w3et[half:half + 64, :])
                # evacuate mix PSUM -> bf16 SBUF (ACT)
                mixs = mxs.tile([128, 4 * NUM_IRREPS], BF16, tag="mixs")
                nc.scalar.activation(
                    mixs[:].rearrange("p (w x) -> p w x", x=192),
                    mixt[:].rearrange("p (w x) -> p w x", x=256)
                    [:, :, 0:192], AF.Copy)
                # A = mix (*) msg0 broadcast over l  (all-bf16 SBUF, 2x)
                nc.vector.tensor_tensor(
                    abv,
                    mixs[:].rearrange("p (w l c) -> p w l c", l=3, c=MUL),
                    m0t[:].rearrange("p (w c) -> p w c", c=MUL)
                        [:, g * 4:g * 4 + nw]
                        .unsqueeze(2).broadcast_to([128, nw, 3, MUL]),
                    OP.mult)
                # tensor products for this half (broadcast ops, no 2x);
                # emitted right after ab(g) so the GPSIMD half launches
                # while DVE handles the g=1 half.
                eng = nc.gpsimd if g == 0 else nc.vector
                mt = mP if g == 0 else mD
                msgv = mt[:].rearrange("p (w f) -> p w f", f=512)
                y8c = y8v[:, wbase + g * 4:wbase + g * 4 + nw]
                eng.tensor_tensor(
                    msgv[:, :, 0:3 * MUL].rearrange(
                        "p w (c k) -> p w c k", k=3),
                    y8c[:, :, 0:3].unsqueeze(2)
                        .broadcast_to([128, nw, MUL, 3]),
                    abv[:, :, 1, :].unsqueeze(3)
                        .broadcast_to([128, nw, MUL, 3]),
                    OP.mult)
                eng.tensor_tensor(
                    msgv[:, :, 3 * MUL:8 * MUL].rearrange(
                        "p w (c k) -> p w c k", k=5),
                    y8c[:, :, 3:8].unsqueeze(2)
                        .broadcast_to([128, nw, MUL, 5]),
                    abv[:, :, 2, :].unsqueeze(3)
                        .broadcast_to([128, nw, MUL, 5]),
                    OP.mult)

        for j in range(CH + LOOKAHEAD):
            jj = j - LOOKAHEAD
            # segment phase for chunk jj first: its inputs are 2 chunks old
            # (ready), and the contiguous matmul block keeps PE ramped.
            for part in range(4):
                seg_part(jj, part)
            if j < CH:
                if j + 1 < CH:
                    prefetch_chunk(j + 1)
                q = strip_of_chunk[j]
                if j == strip_chunks[q][0]:
                    if q + 1 < NSTRIP:
                        prefetch_vt(q + 1)
                    sph_strip(q)
                chunk_body(j)
        # empty blocks (defensive): write zeros
        empty = [b for b in range(BLOCKS) if meta["B_HI"][b] < meta["B_LO"][b]]
        if empty:
            zt = const.tile([128, MSG_W], F32)
            nc.vector.memset(zt[:], 0.0)
            for b in empty:
                nc.sync.dma_start(out_d[b * 128:(b + 1) * 128, :], zt[:])
    nc.compile()
    return nc


def kernel(**inputs) -> np.ndarray:
    in_maps, meta = _prep(**inputs)
    nc = _build(meta)
    from concourse.bass_utils import run_bass_kernel_spmd
    res = run_bass_kernel_spmd(nc, in_maps, list(range(N_CORES)))
    outs = [np.asarray(res.results[c]["out"], np.float32)
            for c in range(N_CORES)]
    return np.concatenate(outs, axis=0)


if __name__ == "__main__":
    import reference
    ins = {k: np.asarray(v) for k, v in reference.setup_inputs().items()}
    out = kernel(**ins)
    exp = np.asarray(reference.reference(**reference.setup_inputs()))
    err = np.abs(out - exp).max() / np.abs(exp).max()
    print("rel err:", err)


# revision 13
# speedup vs baseline: 1.2174x; 1.0558x over previous
"""Trainium2 Bass kernel for MACE-style message-passing convolution.

Reference computation (per edge e with sender s, receiver r):
    msg0 = node_feats[s]                          # [64] scalars
    u    = vectors[e] / |vectors[e]|
    Y1   = sqrt(3) u ;  Y2 = 5 quadratic harmonics of u
    mix  = MLP(radial[e])                         # [192] = m0|m1|m2
    msg  = [msg0*m0, (msg0 (x) Y1)*m1, (msg0 (x) Y2)*m2]   # [576]
    out[r] += msg / sqrt(16)

Strategy (8 NeuronCores, SPMD):
  * Host: sort edges by receiver, shard NODES across cores (core c owns
    nodes [2048c, 2048(c+1))) so each core gets a contiguous slice of
    sorted edges -> no collective needed.
  * Host bakes data layouts only (gather of node_feats by sender, edge
    window layout, one-hot scatter matrices, weight folding). All FLOPs
    of the reference run on-device.
  * Device per core, tuned against the HW perfetto trace:
      - MLP on PE in fp32r (full rate at 512-wide moving dim), silu on ACT.
      - mix matmul in bf16 (fp32r at 256-wide runs 1/4 rate on silicon).
      - ab = mix*msg0 on DVE (all-bf16 SBUF operands -> 2x mode).
      - tensor products (broadcast ops, no 2x possible) split between
        DVE and the otherwise-idle GPSIMD engine.
      - segment-sum via one-hot bf16 matmuls accumulating in a single
        [512|64] PSUM tile per 128-node block; one ACT evacuation
        instruction per block + 2 output DMAs.
      - chunk pipeline software-pipelined 2 deep.
"""

import os
import sys
from contextlib import ExitStack

import numpy as np

sys.path.insert(0, "/opt/trn_rl_repo")

import ml_dtypes  # noqa: E402

import concourse.bass as bass  # noqa: E402
import concourse.bacc as bacc  # noqa: E402
import concourse.tile as tile  # noqa: E402
from concourse import mybir  # noqa: E402

N_CORES = 8
N_NODES = 16384
N_EDGES = 262144
MUL = 64
N_BASIS = 8
HIDDEN = 64
NUM_IRREPS = 3 * MUL  # 192
MSG_W = 9 * MUL  # 576
NODES_PER_CORE = N_NODES // N_CORES  # 2048
BLOCKS = NODES_PER_CORE // 128  # 16 node-blocks of 128
WIN = 128  # edges per window (matmul K)
CHUNK_E = 1024  # edges per MLP chunk (2 stacked groups of 512)

# Tensor products: GPSIMD owns windows [0,4) (fed by the g=0 half-chunk),
# DVE owns windows [4,8) (fed by g=1). Separate msg/ab tiles per half so
# the two engines never touch the same tile (the tile framework serializes
# cross-engine writers of one tile). GPSIMD handles full-width contiguous
# runs at ~1.8ns/col but chokes on column-sliced APs.
POOL_W = 4

F32 = mybir.dt.float32
BF16 = mybir.dt.bfloat16
F32R = mybir.dt.float32r
AF = mybir.ActivationFunctionType
OP = mybir.AluOpType


def _silu_norm():
    x = np.linspace(-12.0, 12.0, 24001)
    p = np.exp(-0.5 * x * x) / np.sqrt(2.0 * np.pi)
    s = x / (1.0 + np.exp(-x))
    trapz = getattr(np, "trapz", None) or np.trapezoid
    return float(1.0 / np.sqrt(trapz(s * s * p, x)))


def _prep(vectors, node_feats, radial_embedding, W0, W1, W2, W3,
          senders, receivers):
    """Host-side data marshaling: sort/shard/pad/bake layouts."""
    snd = np.asarray(senders).astype(np.int64)
    rcv = np.asarray(receivers).astype(np.int64)
    vectors = np.asarray(vectors, dtype=np.float32)
    node_feats = np.asarray(node_feats, dtype=np.float32)
    radial = np.asarray(radial_embedding, dtype=np.float32)

    perm = np.argsort(rcv, kind="stable")
    rcv_s = rcv[perm]
    snd_s = snd[perm]
    v_s = vectors[perm]
    rad_s = radial[perm]

    bounds = np.searchsorted(rcv_s, np.arange(N_CORES + 1) * NODES_PER_CORE)
    e_counts = np.diff(bounds)
    E_pad = int(np.ceil(e_counts.max() / CHUNK_E) * CHUNK_E)
    W = E_pad // WIN  # windows per core
    CH = E_pad // CHUNK_E

    sn = _silu_norm()
    W0e = (np.asarray(W0, np.float32) / np.sqrt(N_BASIS))
    W1e = (np.asarray(W1, np.float32) * sn / np.sqrt(HIDDEN))
    W2e = (np.asarray(W2, np.float32) * sn / np.sqrt(HIDDEN))
    W3e = (np.asarray(W3, np.float32) * sn / np.sqrt(HIDDEN) / 4.0).copy()
    W3e[:, MUL:2 * MUL] *= np.sqrt(3.0)  # fold Y1 = sqrt(3) u

    def blockdiag(w):
        k, m = w.shape
        out = np.zeros((2 * k, 2 * m), np.float32)
        out[:k, :m] = w
        out[k:, m:] = w
        return out

    w01 = blockdiag(W0e)
    w1b = blockdiag(W1e)
    w2b = blockdiag(W2e)
    w3e = W3e.astype(ml_dtypes.bfloat16)  # [64, 192] bf16

    # Per-core block->window ranges, unified across cores (SPMD: one program)
    core = {}
    blo_all = np.full((N_CORES, BLOCKS), 10**9, np.int64)
    bhi_all = np.full((N_CORES, BLOCKS), -1, np.int64)
    for c in range(N_CORES):
        lo, hi = bounds[c], bounds[c + 1]
        ec = hi - lo
        rl = rcv_s[lo:hi] - c * NODES_PER_CORE  # local node ids [0, 2048)
        rl_pad = np.full(E_pad, -1, np.int64)
        rl_pad[:ec] = rl
        # block edge ranges within this core's (padded) edge list
        bb = np.searchsorted(rl, np.arange(BLOCKS + 1) * 128)
        for b in range(BLOCKS):
            if bb[b + 1] > bb[b]:
                blo_all[c, b] = bb[b] // WIN
                bhi_all[c, b] = (bb[b + 1] - 1) // WIN
        core[c] = dict(lo=lo, hi=hi, ec=ec, rl_pad=rl_pad)
    B_LO = blo_all.min(axis=0)
    B_HI = bhi_all.max(axis=0)
    for b in range(BLOCKS):
        if B_HI[b] < B_LO[b]:
            B_LO[b], B_HI[b] = 0, -1  # empty everywhere -> memset path
    # pair list in window-major emission order
    pairs = []  # (w, b, start, stop)
    for w in range(W):
        for b in range(BLOCKS):
            if B_LO[b] <= w <= B_HI[b]:
                pairs.append((w, b, w == B_LO[b], w == B_HI[b]))
    n_pairs = len(pairs)

    in_maps = []
    for c in range(N_CORES):
        cc = core[c]
        lo, ec = cc["lo"], cc["ec"]
        # padded per-core edge arrays
        v_pad = np.zeros((E_pad, 3), np.float32)
        v_pad[:, 0] = 1.0
        v_pad[:ec] = v_s[lo:lo + ec]
        rad_pad = np.zeros((E_pad, N_BASIS), np.float32)
        rad_pad[:ec] = rad_s[lo:lo + ec]
        snd_pad = np.zeros(E_pad, np.int64)
        snd_pad[:ec] = snd_s[lo:lo + ec]

        msg0 = node_feats[snd_pad]  # [E_pad, 64] host gather (layout only)
        msg0 = (msg0.reshape(W, WIN, MUL).transpose(1, 0, 2)
                .reshape(128, W * MUL).astype(ml_dtypes.bfloat16))

        vint = v_pad.reshape(W, WIN, 3).transpose(1, 0, 2).reshape(128, 3 * W)

        r4 = rad_pad.reshape(CH, 2, 512, N_BASIS)
        rad16 = np.ascontiguousarray(
            r4.transpose(1, 3, 0, 2).reshape(16, CH * 512))

        # one-hot scatter matrices per (w, b) pair, bf16 (exact 0/1)
        rlp = cc["rl_pad"]
        ohs = np.zeros((n_pairs, WIN, 128), ml_dtypes.bfloat16)
        ar = np.arange(128)
        for i, (w, b, _, _) in enumerate(pairs):
            rloc = rlp[w * WIN:(w + 1) * WIN] - 128 * b
            ohs[i] = (rloc[:, None] == ar[None, :]).astype(ml_dtypes.bfloat16)
        ohs = ohs.transpose(1, 0, 2).reshape(WIN, n_pairs * 128)

        in_maps.append({
            "msg0": np.ascontiguousarray(msg0),
            "vint": np.ascontiguousarray(vint),
            "rad16": np.ascontiguousarray(rad16),
            "ohs": np.ascontiguousarray(ohs),
            "w01": w01, "w1b": w1b, "w2b": w2b, "w3e": w3e,
        })

    meta = dict(W=W, CH=CH, pairs=pairs, n_pairs=n_pairs,
                B_LO=B_LO, B_HI=B_HI)
    return in_maps, meta


def _build(meta, sim_safe=False):
    """Build the SPMD Bass/Tile program (identical across cores).

    sim_safe: CoreSim doesn't implement the Silu ACT function; emit
    Sigmoid + elementwise multiply instead (identical math) for sim runs.
    """
    W = meta["W"]
    CH = meta["CH"]
    pairs = meta["pairs"]
    n_pairs = meta["n_pairs"]

    FR = F32 if sim_safe else F32R
    nc = bacc.Bacc("TRN2", target_bir_lowering=False, debug=False)
    msg0_d = nc.declare_dram_parameter("msg0", [128, W * MUL], BF16, isOutput=False)
    vint_d = nc.declare_dram_parameter("vint", [128, 3 * W], F32, isOutput=False)
    rad_d = nc.declare_dram_parameter("rad16", [16, CH * 512], FR, isOutput=False)
    ohs_d = nc.declare_dram_parameter("ohs", [128, n_pairs * 128], BF16, isOutput=False)
    w01_d = nc.declare_dram_parameter("w01", [16, 128], FR, isOutput=False)
    w1b_d = nc.declare_dram_parameter("w1b", [128, 128], FR, isOutput=False)
    w2b_d = nc.declare_dram_parameter("w2b", [128, 128], FR, isOutput=False)
    w3e_d = nc.declare_dram_parameter("w3e", [64, NUM_IRREPS], BF16, isOutput=False)
    out_d = nc.declare_dram_parameter("out", [NODES_PER_CORE, MSG_W], F32,
                                      isOutput=True)

    C15 = float(np.sqrt(15.0))
    C5H = float(np.sqrt(5.0) / 2.0)

    def silu(out_ap, in_ap):
        if sim_safe:
            nc.scalar.activation(out_ap, in_ap, AF.Sigmoid)
            nc.vector.tensor_tensor(out_ap, out_ap, in_ap, OP.mult)
        else:
            nc.scalar.activation(out_ap, in_ap, AF.Silu)

    assert POOL_W == 4  # mix/ab half-chunks are fixed 4-window groups
    # sph strips: ~CH/4 chunks each, chunk-aligned
    NSTRIP = min(4, CH)
    base_sz, rem = divmod(CH, NSTRIP)
    strip_chunks = []  # (chunk_lo, chunk_hi)
    c0 = 0
    for q in range(NSTRIP):
        sz = base_sz + (1 if q < rem else 0)
        strip_chunks.append((c0, c0 + sz))
        c0 += sz
    strip_of_chunk = {}
    for q, (lo, hi) in enumerate(strip_chunks):
        for j in range(lo, hi):
            strip_of_chunk[j] = q

    with tile.TileContext(nc) as tc, ExitStack() as ctx:
        const = ctx.enter_context(tc.tile_pool(name="const", bufs=1))
        sphp = ctx.enter_context(tc.tile_pool(name="sph", bufs=2))
        y8p = ctx.enter_context(tc.tile_pool(name="y8p", bufs=1))
        radp = ctx.enter_context(tc.tile_pool(name="rad", bufs=2))
        hp = ctx.enter_context(tc.tile_pool(name="hp", bufs=2, space="PSUM"))
        hact = ctx.enter_context(tc.tile_pool(name="hact", bufs=2))
        h3p = ctx.enter_context(tc.tile_pool(name="h3", bufs=2))
        mixp = ctx.enter_context(tc.tile_pool(name="mixp", bufs=1, space="PSUM"))
        mxs = ctx.enter_context(tc.tile_pool(name="mxs", bufs=2))
        abp = ctx.enter_context(tc.tile_pool(name="ab", bufs=8))
        m0p = ctx.enter_context(tc.tile_pool(name="m0", bufs=4))
        msgp = ctx.enter_context(tc.tile_pool(name="msg", bufs=8))
        ohp = ctx.enter_context(tc.tile_pool(name="oh", bufs=3))
        aggp = ctx.enter_context(tc.tile_pool(name="agg", bufs=2, space="PSUM"))
        outp = ctx.enter_context(tc.tile_pool(name="outs", bufs=2))

        LOOKAHEAD = 2
        rad_tiles = {}
        m0_tiles = {}

        def prefetch_chunk(j):
            radt = radp.tile([16, 512], FR, tag="radt")
            nc.sync.dma_start(radt[:], rad_d[:, j * 512:(j + 1) * 512])
            rad_tiles[j] = radt
            m0t = m0p.tile([128, 8 * MUL], BF16, tag="m0t")
            nc.sync.dma_start(
                m0t[:], msg0_d[:, j * 8 * MUL:(j + 1) * 8 * MUL])
            m0_tiles[j] = m0t

        # chunk 0 inputs + weights first so PE can start immediately
        prefetch_chunk(0)
        w01t = const.tile([16, 128], FR)
        nc.sync.dma_start(w01t[:], w01_d[:])
        w1bt = const.tile([128, 128], FR)
        nc.sync.dma_start(w1bt[:], w1b_d[:])
        w2bt = const.tile([128, 128], FR)
        nc.sync.dma_start(w2bt[:], w2b_d[:])
        # two copies of W3 (partitions 0:64 and 64:128) so the mix matmul's
        # lhsT (h3 slice) and rhs share a base partition
        w3et = const.tile([128, NUM_IRREPS], BF16)
        nc.sync.dma_start(w3et[0:64, :], w3e_d[:])
        nc.sync.dma_start(w3et[64:128, :], w3e_d[:])

        # y8[q] [128, Wq, 8] bf16: per window cols [u_x u_y u_z | y2_0..y2_4]
        y8_tiles = {}
        vt_tiles = {}

        def prefetch_vt(q):
            lo, hi = strip_chunks[q]
            Wq = (hi - lo) * 8
            vt = sphp.tile([128, 3 * Wq], F32, tag="vt", name=f"vt{q}")
            nc.sync.dma_start(vt[:], vint_d[:, lo * 24:lo * 24 + 3 * Wq])
            vt_tiles[q] = vt

        def sph_strip(q):
            lo, hi = strip_chunks[q]
            Wq = (hi - lo) * 8
            vt = vt_tiles[q]
            vsq = sphp.tile([128, 3 * Wq], F32, tag="vsq")
            nc.vector.tensor_tensor(vsq[:], vt[:], vt[:], OP.mult)
            s2 = sphp.tile([128, Wq], F32, tag="s2")
            nc.vector.tensor_reduce(
                s2[:], vsq[:].rearrange("p (w k) -> p w k", k=3),
                mybir.AxisListType.X, OP.add)
            rs = sphp.tile([128, Wq], F32, tag="rs")
            nc.vector.reciprocal(rs[:], s2[:])
            rinv = sphp.tile([128, Wq], F32, tag="rinv")  # 1/|v|
            nc.scalar.activation(rinv[:], rs[:], AF.Sqrt)
            u3 = sphp.tile([128, 3 * Wq], F32, tag="u3")
            nc.vector.tensor_tensor(
                u3[:].rearrange("p (w k) -> p w k", k=3),
                vt[:].rearrange("p (w k) -> p w k", k=3),
                rinv[:].unsqueeze(2).broadcast_to([128, Wq, 3]),
                OP.mult)
            ux = u3[:].rearrange("p (w k) -> p k w", k=3)[:, 0]
            uy = u3[:].rearrange("p (w k) -> p k w", k=3)[:, 1]
            uz = u3[:].rearrange("p (w k) -> p k w", k=3)[:, 2]
            y5 = sphp.tile([128, 5 * Wq], F32, tag="y5")
            y5v = y5[:].rearrange("p (w k) -> p k w", k=5)
            nc.vector.scalar_tensor_tensor(y5v[:, 0], ux, C15, uy,
                                           OP.mult, OP.mult)
            nc.vector.scalar_tensor_tensor(y5v[:, 1], uy, C15, uz,
                                           OP.mult, OP.mult)
            nc.vector.scalar_tensor_tensor(y5v[:, 2], uz, 3.0 * C5H, uz,
                                           OP.mult, OP.mult)
            nc.vector.tensor_scalar_add(y5v[:, 2], y5v[:, 2], -C5H)
            nc.vector.scalar_tensor_tensor(y5v[:, 3], ux, C15, uz,
                                           OP.mult, OP.mult)
            tpq = sphp.tile([128, 2 * Wq], F32, tag="tpq")
            nc.vector.tensor_tensor(tpq[:, :Wq], ux, uy, OP.add)
            nc.vector.tensor_tensor(tpq[:, Wq:], ux, uy, OP.subtract)
            nc.vector.scalar_tensor_tensor(y5v[:, 4], tpq[:, :Wq], C15 / 2.0,
                                           tpq[:, Wq:], OP.mult, OP.mult)
            y8 = y8p.tile([128, Wq * 8], BF16, name=f"y8_{q}")
            y8_tiles[q] = y8
            y8v = y8[:].rearrange("p (w k) -> p w k", k=8)
            nc.vector.tensor_copy(
                y8v[:, :, 0:3], u3[:].rearrange("p (w k) -> p w k", k=3))
            nc.vector.tensor_copy(
                y8v[:, :, 3:8],
                y5[:].rearrange("p (w k) -> p w k", k=5))

        prefetch_vt(0)

        # segment bookkeeping: pairs grouped by msg chunk, split in 4 parts
        chunk_pair_rng = {}
        for i, (w, b, _, _) in enumerate(pairs):
            jj = w // 8
            lo, hi = chunk_pair_rng.get(jj, (i, i))
            chunk_pair_rng[jj] = (min(lo, i), i + 1)

        pair_i = 0
        agg_t = {}
        ab_tiles = {}
        msg_tiles = {}
        oh_tiles = {}

        def prefetch_oh(gi):
            if gi in oh_tiles or gi * 8 >= n_pairs:
                return
            oht = ohp.tile([128, 8 * 128], BF16, tag="oh", bufs=3)
            n_in = min(8 * 128, (n_pairs - gi * 8) * 128)
            nc.sync.dma_start(
                oht[:, :n_in], ohs_d[:, gi * 8 * 128:gi * 8 * 128 + n_in])
            oh_tiles[gi] = oht

        def seg_part(jj, take=None):
            nonlocal pair_i
            if jj < 0 or jj >= CH or jj not in chunk_pair_rng:
                return
            lo, hi = chunk_pair_rng[jj]
            stop_i = hi if take is None else min(hi, pair_i + take)
            mP, mD = msg_tiles[jj]
            msgvP = mP[:].rearrange("p (w f) -> p w f", f=512)
            msgvD = mD[:].rearrange("p (w f) -> p w f", f=512)
            aP, aD = ab_tiles[jj]
            abvP = aP[:].rearrange("p (w l c) -> p w l c", l=3, c=MUL)
            abvD = aD[:].rearrange("p (w l c) -> p w l c", l=3, c=MUL)
            while pair_i < stop_i:
                w, b, is_start, is_stop = pairs[pair_i]
                wj = w % 8
                gi, gs = divmod(pair_i, 8)
                prefetch_oh(gi)
                prefetch_oh(gi + 1)
                if is_start:
                    # single PSUM tile: [0:512] = l1|l2, [512:576] = l0
                    agg_t[b] = aggp.tile([128, 640], F32, tag="agg",
                                         name=f"agg{b}")
                at = agg_t[b]
                lhs = oh_tiles[gi][:, gs * 128:(gs + 1) * 128]
                rhs_b = (msgvP[:, wj, :] if wj < POOL_W
                         else msgvD[:, wj - POOL_W, :])
                rhs_a = (abvP[:, wj, 0, :] if wj < POOL_W
                         else abvD[:, wj - POOL_W, 0, :])
                nc.tensor.matmul(at[:, 0:512], lhs, rhs_b,
                                 start=is_start, stop=is_stop)
                nc.tensor.matmul(at[:, 512:576], lhs, rhs_a,
                                 start=is_start, stop=is_stop)
                if is_stop:
                    ot = outp.tile([128, MSG_W], F32, tag="ot")
                    nc.scalar.activation(ot[:], at[:, 0:576], AF.Copy)
                    nc.sync.dma_start(
                        out_d[b * 128:(b + 1) * 128, 0:MUL],
                        ot[:, 512:576])
                    nc.sync.dma_start(
                        out_d[b * 128:(b + 1) * 128, MUL:MSG_W],
                        ot[:, 0:512])
                pair_i += 1

        h1_tiles = {}
        h2_tiles = {}
        h3_tiles = {}

        def mlp_h1(j):
            h1ps = hp.tile([128, 512], F32, tag="hps")
            nc.tensor.matmul(h1ps[:], w01t[:], rad_tiles[j][:])
            h1 = hact.tile([128, 512], FR, tag="h1t")
            silu(h1[:], h1ps[:])
            h1_tiles[j] = h1

        def mlp_h2(j):
            h2ps = hp.tile([128, 512], F32, tag="hps")
            nc.tensor.matmul(h2ps[:], w1bt[:], h1_tiles.pop(j)[:])
            h2 = hact.tile([128, 512], FR, tag="h2t")
            silu(h2[:], h2ps[:])
            h2_tiles[j] = h2

        def mlp_h3(j):
            h3ps = hp.tile([128, 512], F32, tag="hps")
            nc.tensor.matmul(h3ps[:], w2bt[:], h2_tiles.pop(j)[:])
            h3 = h3p.tile([128, 512], BF16)
            silu(h3[:], h3ps[:])
            h3_tiles[j] = h3

        def chunk_tail(j, js):
            # mix + ab + tensor products; GPSIMD owns windows 0:4 (g=0),
            # DVE owns windows 4:8 (g=1), with disjoint msg/ab tiles so the
            # engines run concurrently (one tile per writer engine).
            # js: segment chunk whose pairs get sprinkled between the two
            # mix halves to cover the mixt PSUM turnaround on the PE queue.
            h3 = h3_tiles.pop(j)
            q = strip_of_chunk[j]
            wbase = (j - strip_chunks[q][0]) * 8
            y8v = y8_tiles[q][:].rearrange("p (w k) -> p w k", k=8)
            m0t = m0_tiles[j]
            mP = msgp.tile([128, POOL_W * 512], BF16, tag="msgP",
                           bufs=LOOKAHEAD + 2)
            mD = msgp.tile([128, (8 - POOL_W) * 512], BF16, tag="msgD",
                           bufs=LOOKAHEAD + 2)
            msg_tiles[j] = (mP, mD)
            aP = abp.tile([128, POOL_W * NUM_IRREPS], BF16, tag="abP",
                          name=f"abP_{j}", bufs=LOOKAHEAD + 2)
            aD = abp.tile([128, (8 - POOL_W) * NUM_IRREPS], BF16, tag="abD",
                          name=f"abD_{j}", bufs=LOOKAHEAD + 2)
            ab_tiles[j] = (aP, aD)
            for g in range(2):  # half-chunks of 4 windows
                nw = POOL_W if g == 0 else 8 - POOL_W
                abt = aP if g == 0 else aD
                abv = abt[:].rearrange("p (w l c) -> p w l c", l=3, c=MUL)
                # mix: edge-major via stationary-h3 matmuls, bf16 full rate;
                # 256-col PSUM slots so each output stays inside one bank
                mixt = mixp.tile([128, 4 * 256], F32, tag="mixt")
                for t4 in range(4):
                    t = g * 4 + t4
                    half, coff = ((0, t * 128) if t < 4
                                  else (64, (t - 4) * 128))
                    nc.tensor.matmul(
                        mixt[:, t4 * 256:t4 * 256 + NUM_IRREPS],
                        h3[half:half + 64, coff:coff + 128],
                        w3et[half:half + 64, :])
                # evacuate mix PSUM -> bf16 SBUF (ACT)
                mixs = mxs.tile([128, 4 * NUM_IRREPS], BF16, tag="mixs")
                nc.scalar.activation(
                    mixs[:].rearrange("p (w x) -> p w x", x=192),
                    mixt[:].rearrange("p (w x) -> p w x", x=256)
                    [:, :, 0:192], AF.Copy)
                # A = mix (*) msg0 broadcast over l  (all-bf16 SBUF, 2x)
                nc.vector.tensor_tensor(
                    abv,
                    mixs[:].rearrange("p (w l c) -> p w l c", l=3, c=MUL),
                    m0t[:].rearrange("p (w c) -> p w c", c=MUL)
                        [:, g * 4:g * 4 + nw]
                        .unsqueeze(2).broadcast_to([128, nw, 3, MUL]),
                    OP.mult)
                # tensor products for this half (broadcast ops, no 2x);
                # emitted right after ab(g) so the GPSIMD half launches
                # while DVE handles the g=1 half.
                eng = nc.gpsimd if g == 0 else nc.vector
                mt = mP if g == 0 else mD
                msgv = mt[:].rearrange("p (w f) -> p w f", f=512)
                y8c = y8v[:, wbase + g * 4:wbase + g * 4 + nw]
                eng.tensor_tensor(
                    msgv[:, :, 0:3 * MUL].rearrange(
                        "p w (c k) -> p w c k", k=3),
                    y8c[:, :, 0:3].unsqueeze(2)
                        .broadcast_to([128, nw, MUL, 3]),
                    abv[:, :, 1, :].unsqueeze(3)
                        .broadcast_to([128, nw, MUL, 3]),
                    OP.mult)
                eng.tensor_tensor(
                    msgv[:, :, 3 * MUL:8 * MUL].rearrange(
                        "p w (c k) -> p w c k", k=5),
                    y8c[:, :, 3:8].unsqueeze(2)
                        .broadcast_to([128, nw, MUL, 5]),
                    abv[:, :, 2, :].unsqueeze(3)
                        .broadcast_to([128, nw, MUL, 5]),
                    OP.mult)
                if g == 0:
                    # always-ready segment pairs cover the PE wait for the
                    # g=0 mix-PSUM evacuation before the g=1 mix matmuls
                    seg_part(js, take=2)

        # Fully software-pipelined main loop: at iteration j every PE
        # instruction's inputs are >=1 iteration old, so the in-order PE
        # queue never stalls (keeps the PE at its ramped pstate):
        #   h1(j+2) | h2(j+1) | h3(j) | mix/ab/tp(j-1) | segment(j-3)
        strip_starts = {strip_chunks[q][0]: q for q in range(NSTRIP)}
        for j in range(-2, CH + 3):
            if j + 2 in strip_starts and j + 2 > 0:
                prefetch_vt(strip_starts[j + 2])
            if j + 1 in strip_starts:
                sph_strip(strip_starts[j + 1])
            if 0 <= j + 2 < CH:
                prefetch_chunk(j + 2)
                mlp_h1(j + 2)
            if 0 <= j + 1 < CH:
                mlp_h2(j + 1)
            if 0 <= j < CH:
                mlp_h3(j)
            if 0 <= j - 1 < CH:
                chunk_tail(j - 1, j - 3)
            seg_part(j - 3)
        # empty blocks (defensive): write zeros
        empty = [b for b in range(BLOCKS) if meta["B_HI"][b] < meta["B_LO"][b]]
        if empty:
            zt = const.tile([128, MSG_W], F32)
            nc.vector.memset(zt[:], 0.0)
            for b in empty:
                nc.sync.dma_start(out_d[b * 128:(b + 1) * 128, :], zt[:])
    nc.compile()
    return nc


def kernel(**inputs) -> np.ndarray:
    in_maps, meta = _prep(**inputs)
    nc = _build(meta)
    from concourse.bass_utils import run_bass_kernel_spmd
    res = run_bass_kernel_spmd(nc, in_maps, list(range(N_CORES)))
    outs = [np.asarray(res.results[c]["out"], np.float32)
            for c in range(N_CORES)]
    return np.concatenate(outs, axis=0)


if __name__ == "__main__":
    import reference
    ins = {k: np.asarray(v) for k, v in reference.setup_inputs().items()}
    out = kernel(**ins)
    exp = np.asarray(reference.reference(**reference.setup_inputs()))
    err = np.abs(out - exp).max() / np.abs(exp).max()
    print("rel err:", err)


# revision 14
# speedup vs baseline: 1.2530x; 1.0292x over previous
"""Trainium2 Bass kernel for MACE-style message-passing convolution.

Reference computation (per edge e with sender s, receiver r):
    msg0 = node_feats[s]                          # [64] scalars
    u    = vectors[e] / |vectors[e]|
    Y1   = sqrt(3) u ;  Y2 = 5 quadratic harmonics of u
    mix  = MLP(radial[e])                         # [192] = m0|m1|m2
    msg  = [msg0*m0, (msg0 (x) Y1)*m1, (msg0 (x) Y2)*m2]   # [576]
    out[r] += msg / sqrt(16)

Strategy (8 NeuronCores, SPMD):
  * Host: sort edges by receiver, shard NODES across cores (core c owns
    nodes [2048c, 2048(c+1))) so each core gets a contiguous slice of
    sorted edges -> no collective needed.
  * Host bakes data layouts only (gather of node_feats by sender, edge
    window layout, one-hot scatter matrices, weight folding). All FLOPs
    of the reference run on-device.
  * Device per core, tuned against the HW perfetto trace:
      - MLP on PE in fp32r (full rate at 512-wide moving dim), silu on ACT.
      - mix matmul in bf16 (fp32r at 256-wide runs 1/4 rate on silicon).
      - ab = mix*msg0 on DVE (all-bf16 SBUF operands -> 2x mode).
      - tensor products (broadcast ops, no 2x possible) split between
        DVE and the otherwise-idle GPSIMD engine.
      - segment-sum via one-hot bf16 matmuls accumulating in a single
        [512|64] PSUM tile per 128-node block; one ACT evacuation
        instruction per block + 2 output DMAs.
      - chunk pipeline software-pipelined 2 deep.
"""

import os
import sys
from contextlib import ExitStack

import numpy as np

sys.path.insert(0, "/opt/trn_rl_repo")

import ml_dtypes  # noqa: E402

import concourse.bass as bass  # noqa: E402
import concourse.bacc as bacc  # noqa: E402
import concourse.tile as tile  # noqa: E402
from concourse import mybir  # noqa: E402

N_CORES = 8
N_NODES = 16384
N_EDGES = 262144
MUL = 64
N_BASIS = 8
HIDDEN = 64
NUM_IRREPS = 3 * MUL  # 192
MSG_W = 9 * MUL  # 576
NODES_PER_CORE = N_NODES // N_CORES  # 2048
BLOCKS = NODES_PER_CORE // 128  # 16 node-blocks of 128
WIN = 128  # edges per window (matmul K)
CHUNK_E = 1024  # edges per MLP chunk (2 stacked groups of 512)

# Tensor products: GPSIMD owns windows [0,4) (fed by the g=0 half-chunk),
# DVE owns windows [4,8) (fed by g=1). Separate msg/ab tiles per half so
# the two engines never touch the same tile (the tile framework serializes
# cross-engine writers of one tile). GPSIMD handles full-width contiguous
# runs at ~1.8ns/col but chokes on column-sliced APs.
POOL_W = 4

F32 = mybir.dt.float32
BF16 = mybir.dt.bfloat16
F32R = mybir.dt.float32r
AF = mybir.ActivationFunctionType
OP = mybir.AluOpType


def _silu_norm():
    x = np.linspace(-12.0, 12.0, 24001)
    p = np.exp(-0.5 * x * x) / np.sqrt(2.0 * np.pi)
    s = x / (1.0 + np.exp(-x))
    trapz = getattr(np, "trapz", None) or np.trapezoid
    return float(1.0 / np.sqrt(trapz(s * s * p, x)))


def _prep(vectors, node_feats, radial_embedding, W0, W1, W2, W3,
          senders, receivers):
    """Host-side data marshaling: sort/shard/pad/bake layouts."""
    snd = np.asarray(senders).astype(np.int64)
    rcv = np.asarray(receivers).astype(np.int64)
    vectors = np.asarray(vectors, dtype=np.float32)
    node_feats = np.asarray(node_feats, dtype=np.float32)
    radial = np.asarray(radial_embedding, dtype=np.float32)

    perm = np.argsort(rcv, kind="stable")
    rcv_s = rcv[perm]
    snd_s = snd[perm]
    v_s = vectors[perm]
    rad_s = radial[perm]

    bounds = np.searchsorted(rcv_s, np.arange(N_CORES + 1) * NODES_PER_CORE)
    e_counts = np.diff(bounds)
    E_pad = int(np.ceil(e_counts.max() / CHUNK_E) * CHUNK_E)
    W = E_pad // WIN  # windows per core
    CH = E_pad // CHUNK_E

    sn = _silu_norm()
    W0e = (np.asarray(W0, np.float32) / np.sqrt(N_BASIS))
    W1e = (np.asarray(W1, np.float32) * sn / np.sqrt(HIDDEN))
    W2e = (np.asarray(W2, np.float32) * sn / np.sqrt(HIDDEN))
    W3e = (np.asarray(W3, np.float32) * sn / np.sqrt(HIDDEN) / 4.0).copy()
    W3e[:, MUL:2 * MUL] *= np.sqrt(3.0)  # fold Y1 = sqrt(3) u

    def blockdiag(w):
        k, m = w.shape
        out = np.zeros((2 * k, 2 * m), np.float32)
        out[:k, :m] = w
        out[k:, m:] = w
        return out

    w01 = blockdiag(W0e)
    w1b = blockdiag(W1e)
    w2b = blockdiag(W2e)
    w3e = W3e.astype(ml_dtypes.bfloat16)  # [64, 192] bf16

    # Per-core block->window ranges, unified across cores (SPMD: one program)
    core = {}
    blo_all = np.full((N_CORES, BLOCKS), 10**9, np.int64)
    bhi_all = np.full((N_CORES, BLOCKS), -1, np.int64)
    for c in range(N_CORES):
        lo, hi = bounds[c], bounds[c + 1]
        ec = hi - lo
        rl = rcv_s[lo:hi] - c * NODES_PER_CORE  # local node ids [0, 2048)
        rl_pad = np.full(E_pad, -1, np.int64)
        rl_pad[:ec] = rl
        # block edge ranges within this core's (padded) edge list
        bb = np.searchsorted(rl, np.arange(BLOCKS + 1) * 128)
        for b in range(BLOCKS):
            if bb[b + 1] > bb[b]:
                blo_all[c, b] = bb[b] // WIN
                bhi_all[c, b] = (bb[b + 1] - 1) // WIN
        core[c] = dict(lo=lo, hi=hi, ec=ec, rl_pad=rl_pad)
    B_LO = blo_all.min(axis=0)
    B_HI = bhi_all.max(axis=0)
    for b in range(BLOCKS):
        if B_HI[b] < B_LO[b]:
            B_LO[b], B_HI[b] = 0, -1  # empty everywhere -> memset path
    # pair list in window-major emission order
    pairs = []  # (w, b, start, stop)
    for w in range(W):
        for b in range(BLOCKS):
            if B_LO[b] <= w <= B_HI[b]:
                pairs.append((w, b, w == B_LO[b], w == B_HI[b]))
    n_pairs = len(pairs)

    in_maps = []
    for c in range(N_CORES):
        cc = core[c]
        lo, ec = cc["lo"], cc["ec"]
        # padded per-core edge arrays
        v_pad = np.zeros((E_pad, 3), np.float32)
        v_pad[:, 0] = 1.0
        v_pad[:ec] = v_s[lo:lo + ec]
        rad_pad = np.zeros((E_pad, N_BASIS), np.float32)
        rad_pad[:ec] = rad_s[lo:lo + ec]
        snd_pad = np.zeros(E_pad, np.int64)
        snd_pad[:ec] = snd_s[lo:lo + ec]

        msg0 = node_feats[snd_pad]  # [E_pad, 64] host gather (layout only)
        msg0 = (msg0.reshape(W, WIN, MUL).transpose(1, 0, 2)
                .reshape(128, W * MUL).astype(ml_dtypes.bfloat16))

        vint = v_pad.reshape(W, WIN, 3).transpose(1, 0, 2).reshape(128, 3 * W)

        r4 = rad_pad.reshape(CH, 2, 512, N_BASIS)
        rad16 = np.ascontiguousarray(
            r4.transpose(1, 3, 0, 2).reshape(16, CH * 512))

        # one-hot scatter matrices per (w, b) pair, bf16 (exact 0/1)
        rlp = cc["rl_pad"]
        ohs = np.zeros((n_pairs, WIN, 128), ml_dtypes.bfloat16)
        ar = np.arange(128)
        for i, (w, b, _, _) in enumerate(pairs):
            rloc = rlp[w * WIN:(w + 1) * WIN] - 128 * b
            ohs[i] = (rloc[:, None] == ar[None, :]).astype(ml_dtypes.bfloat16)
        ohs = ohs.transpose(1, 0, 2).reshape(WIN, n_pairs * 128)

        in_maps.append({
            "msg0": np.ascontiguousarray(msg0),
            "vint": np.ascontiguousarray(vint),
            "rad16": np.ascontiguousarray(rad16),
            "ohs": np.ascontiguousarray(ohs),
            "w01": w01, "w1b": w1b, "w2b": w2b, "w3e": w3e,
        })

    meta = dict(W=W, CH=CH, pairs=pairs, n_pairs=n_pairs,
                B_LO=B_LO, B_HI=B_HI)
    return in_maps, meta


def _build(meta, sim_safe=False):
    """Build the SPMD Bass/Tile program (identical across cores).

    sim_safe: CoreSim doesn't implement the Silu ACT function; emit
    Sigmoid + elementwise multiply instead (identical math) for sim runs.
    """
    W = meta["W"]
    CH = meta["CH"]
    pairs = meta["pairs"]
    n_pairs = meta["n_pairs"]

    FR = F32 if sim_safe else F32R
    nc = bacc.Bacc("TRN2", target_bir_lowering=False, debug=False)
    msg0_d = nc.declare_dram_parameter("msg0", [128, W * MUL], BF16, isOutput=False)
    vint_d = nc.declare_dram_parameter("vint", [128, 3 * W], F32, isOutput=False)
    rad_d = nc.declare_dram_parameter("rad16", [16, CH * 512], FR, isOutput=False)
    ohs_d = nc.declare_dram_parameter("ohs", [128, n_pairs * 128], BF16, isOutput=False)
    w01_d = nc.declare_dram_parameter("w01", [16, 128], FR, isOutput=False)
    w1b_d = nc.declare_dram_parameter("w1b", [128, 128], FR, isOutput=False)
    w2b_d = nc.declare_dram_parameter("w2b", [128, 128], FR, isOutput=False)
    w3e_d = nc.declare_dram_parameter("w3e", [64, NUM_IRREPS], BF16, isOutput=False)
    out_d = nc.declare_dram_parameter("out", [NODES_PER_CORE, MSG_W], F32,
                                      isOutput=True)

    C15 = float(np.sqrt(15.0))
    C5H = float(np.sqrt(5.0) / 2.0)

    def silu(out_ap, in_ap):
        if sim_safe:
            nc.scalar.activation(out_ap, in_ap, AF.Sigmoid)
            nc.vector.tensor_tensor(out_ap, out_ap, in_ap, OP.mult)
        else:
            nc.scalar.activation(out_ap, in_ap, AF.Silu)

    assert POOL_W == 4  # mix/ab half-chunks are fixed 4-window groups
    # sph strips: ~CH/4 chunks each, chunk-aligned
    NSTRIP = min(4, CH)
    base_sz, rem = divmod(CH, NSTRIP)
    strip_chunks = []  # (chunk_lo, chunk_hi)
    c0 = 0
    for q in range(NSTRIP):
        sz = base_sz + (1 if q < rem else 0)
        strip_chunks.append((c0, c0 + sz))
        c0 += sz
    strip_of_chunk = {}
    for q, (lo, hi) in enumerate(strip_chunks):
        for j in range(lo, hi):
            strip_of_chunk[j] = q

    with tile.TileContext(nc) as tc, ExitStack() as ctx:
        const = ctx.enter_context(tc.tile_pool(name="const", bufs=1))
        sphp = ctx.enter_context(tc.tile_pool(name="sph", bufs=2))
        y8p = ctx.enter_context(tc.tile_pool(name="y8p", bufs=1))
        radp = ctx.enter_context(tc.tile_pool(name="rad", bufs=2))
        hp = ctx.enter_context(tc.tile_pool(name="hp", bufs=2, space="PSUM"))
        hact = ctx.enter_context(tc.tile_pool(name="hact", bufs=2))
        h3p = ctx.enter_context(tc.tile_pool(name="h3", bufs=2))
        mixp = ctx.enter_context(tc.tile_pool(name="mixp", bufs=1, space="PSUM"))
        mxs = ctx.enter_context(tc.tile_pool(name="mxs", bufs=2))
        abp = ctx.enter_context(tc.tile_pool(name="ab", bufs=8))
        m0p = ctx.enter_context(tc.tile_pool(name="m0", bufs=4))
        msgp = ctx.enter_context(tc.tile_pool(name="msg", bufs=8))
        ohp = ctx.enter_context(tc.tile_pool(name="oh", bufs=3))
        aggp = ctx.enter_context(tc.tile_pool(name="agg", bufs=2, space="PSUM"))
        outp = ctx.enter_context(tc.tile_pool(name="outs", bufs=2))

        LOOKAHEAD = 2
        rad_tiles = {}
        m0_tiles = {}

        def prefetch_chunk(j):
            radt = radp.tile([16, 512], FR, tag="radt")
            nc.sync.dma_start(radt[:], rad_d[:, j * 512:(j + 1) * 512])
            rad_tiles[j] = radt
            m0t = m0p.tile([128, 8 * MUL], BF16, tag="m0t")
            nc.sync.dma_start(
                m0t[:], msg0_d[:, j * 8 * MUL:(j + 1) * 8 * MUL])
            m0_tiles[j] = m0t

        # chunk 0 inputs + weights first so PE can start immediately
        prefetch_chunk(0)
        w01t = const.tile([16, 128], FR)
        nc.sync.dma_start(w01t[:], w01_d[:])
        w1bt = const.tile([128, 128], FR)
        nc.sync.dma_start(w1bt[:], w1b_d[:])
        w2bt = const.tile([128, 128], FR)
        nc.sync.dma_start(w2bt[:], w2b_d[:])
        # two copies of W3 (partitions 0:64 and 64:128) so the mix matmul's
        # lhsT (h3 slice) and rhs share a base partition
        w3et = const.tile([128, NUM_IRREPS], BF16)
        nc.sync.dma_start(w3et[0:64, :], w3e_d[:])
        nc.sync.dma_start(w3et[64:128, :], w3e_d[:])

        # y8[q] [128, Wq, 8] bf16: per window cols [u_x u_y u_z | y2_0..y2_4]
        y8_tiles = {}
        vt_tiles = {}

        def prefetch_vt(q):
            lo, hi = strip_chunks[q]
            Wq = (hi - lo) * 8
            vt = sphp.tile([128, 3 * Wq], F32, tag="vt", name=f"vt{q}")
            nc.sync.dma_start(vt[:], vint_d[:, lo * 24:lo * 24 + 3 * Wq])
            vt_tiles[q] = vt

        def sph_strip(q):
            lo, hi = strip_chunks[q]
            Wq = (hi - lo) * 8
            vt = vt_tiles[q]
            vsq = sphp.tile([128, 3 * Wq], F32, tag="vsq")
            nc.vector.tensor_tensor(vsq[:], vt[:], vt[:], OP.mult)
            s2 = sphp.tile([128, Wq], F32, tag="s2")
            nc.vector.tensor_reduce(
                s2[:], vsq[:].rearrange("p (w k) -> p w k", k=3),
                mybir.AxisListType.X, OP.add)
            rs = sphp.tile([128, Wq], F32, tag="rs")
            nc.vector.reciprocal(rs[:], s2[:])
            rinv = sphp.tile([128, Wq], F32, tag="rinv")  # 1/|v|
            nc.scalar.activation(rinv[:], rs[:], AF.Sqrt)
            u3 = sphp.tile([128, 3 * Wq], F32, tag="u3")
            nc.vector.tensor_tensor(
                u3[:].rearrange("p (w k) -> p w k", k=3),
                vt[:].rearrange("p (w k) -> p w k", k=3),
                rinv[:].unsqueeze(2).broadcast_to([128, Wq, 3]),
                OP.mult)
            ux = u3[:].rearrange("p (w k) -> p k w", k=3)[:, 0]
            uy = u3[:].rearrange("p (w k) -> p k w", k=3)[:, 1]
            uz = u3[:].rearrange("p (w k) -> p k w", k=3)[:, 2]
            y5 = sphp.tile([128, 5 * Wq], F32, tag="y5")
            y5v = y5[:].rearrange("p (w k) -> p k w", k=5)
            nc.vector.scalar_tensor_tensor(y5v[:, 0], ux, C15, uy,
                                           OP.mult, OP.mult)
            nc.vector.scalar_tensor_tensor(y5v[:, 1], uy, C15, uz,
                                           OP.mult, OP.mult)
            nc.vector.scalar_tensor_tensor(y5v[:, 2], uz, 3.0 * C5H, uz,
                                           OP.mult, OP.mult)
            nc.vector.tensor_scalar_add(y5v[:, 2], y5v[:, 2], -C5H)
            nc.vector.scalar_tensor_tensor(y5v[:, 3], ux, C15, uz,
                                           OP.mult, OP.mult)
            tpq = sphp.tile([128, 2 * Wq], F32, tag="tpq")
            nc.vector.tensor_tensor(tpq[:, :Wq], ux, uy, OP.add)
            nc.vector.tensor_tensor(tpq[:, Wq:], ux, uy, OP.subtract)
            nc.vector.scalar_tensor_tensor(y5v[:, 4], tpq[:, :Wq], C15 / 2.0,
                                           tpq[:, Wq:], OP.mult, OP.mult)
            y8 = y8p.tile([128, Wq * 8], BF16, name=f"y8_{q}")
            y8_tiles[q] = y8
            y8v = y8[:].rearrange("p (w k) -> p w k", k=8)
            nc.vector.tensor_copy(
                y8v[:, :, 0:3], u3[:].rearrange("p (w k) -> p w k", k=3))
            nc.vector.tensor_copy(
                y8v[:, :, 3:8],
                y5[:].rearrange("p (w k) -> p w k", k=5))

        prefetch_vt(0)

        # segment bookkeeping: pairs grouped by msg chunk, split in 4 parts
        chunk_pair_rng = {}
        for i, (w, b, _, _) in enumerate(pairs):
            jj = w // 8
            lo, hi = chunk_pair_rng.get(jj, (i, i))
            chunk_pair_rng[jj] = (min(lo, i), i + 1)

        pair_i = 0
        agg_t = {}
        ab_tiles = {}
        msg_tiles = {}
        oh_tiles = {}

        def prefetch_oh(gi):
            if gi in oh_tiles or gi * 8 >= n_pairs:
                return
            oht = ohp.tile([128, 8 * 128], BF16, tag="oh", bufs=3)
            n_in = min(8 * 128, (n_pairs - gi * 8) * 128)
            nc.sync.dma_start(
                oht[:, :n_in], ohs_d[:, gi * 8 * 128:gi * 8 * 128 + n_in])
            oh_tiles[gi] = oht

        def seg_part(jj, take=None):
            nonlocal pair_i
            if jj < 0 or jj >= CH or jj not in chunk_pair_rng:
                return
            lo, hi = chunk_pair_rng[jj]
            stop_i = hi if take is None else min(hi, pair_i + take)
            mP, mD = msg_tiles[jj]
            msgvP = mP[:].rearrange("p (w f) -> p w f", f=512)
            msgvD = mD[:].rearrange("p (w f) -> p w f", f=512)
            aP, aD = ab_tiles[jj]
            abvP = aP[:].rearrange("p (w l c) -> p w l c", l=3, c=MUL)
            abvD = aD[:].rearrange("p (w l c) -> p w l c", l=3, c=MUL)
            while pair_i < stop_i:
                w, b, is_start, is_stop = pairs[pair_i]
                wj = w % 8
                gi, gs = divmod(pair_i, 8)
                prefetch_oh(gi)
                prefetch_oh(gi + 1)
                if is_start:
                    # single PSUM tile: [0:512] = l1|l2, [512:576] = l0
                    agg_t[b] = aggp.tile([128, 640], F32, tag="agg",
                                         name=f"agg{b}")
                at = agg_t[b]
                lhs = oh_tiles[gi][:, gs * 128:(gs + 1) * 128]
                rhs_b = (msgvP[:, wj, :] if wj < POOL_W
                         else msgvD[:, wj - POOL_W, :])
                rhs_a = (abvP[:, wj, 0, :] if wj < POOL_W
                         else abvD[:, wj - POOL_W, 0, :])
                nc.tensor.matmul(at[:, 0:512], lhs, rhs_b,
                                 start=is_start, stop=is_stop)
                nc.tensor.matmul(at[:, 512:576], lhs, rhs_a,
                                 start=is_start, stop=is_stop)
                if is_stop:
                    ot = outp.tile([128, MSG_W], F32, tag="ot")
                    nc.scalar.activation(ot[:], at[:, 0:576], AF.Copy)
                    nc.sync.dma_start(
                        out_d[b * 128:(b + 1) * 128, 0:MUL],
                        ot[:, 512:576])
                    nc.sync.dma_start(
                        out_d[b * 128:(b + 1) * 128, MUL:MSG_W],
                        ot[:, 0:512])
                pair_i += 1

        h1_tiles = {}
        h2_tiles = {}
        h3_tiles = {}

        def mlp_h1(j):
            h1ps = hp.tile([128, 512], F32, tag="hps")
            nc.tensor.matmul(h1ps[:], w01t[:], rad_tiles[j][:])
            h1 = hact.tile([128, 512], FR, tag="h1t")
            silu(h1[:], h1ps[:])
            h1_tiles[j] = h1

        def mlp_h2(j):
            h2ps = hp.tile([128, 512], F32, tag="hps")
            nc.tensor.matmul(h2ps[:], w1bt[:], h1_tiles.pop(j)[:])
            h2 = hact.tile([128, 512], FR, tag="h2t")
            silu(h2[:], h2ps[:])
            h2_tiles[j] = h2

        def mlp_h3(j):
            h3ps = hp.tile([128, 512], F32, tag="hps")
            nc.tensor.matmul(h3ps[:], w2bt[:], h2_tiles.pop(j)[:])
            h3 = h3p.tile([128, 512], BF16)
            silu(h3[:], h3ps[:])
            h3_tiles[j] = h3

        def mix_g(j, g):
            # mix matmuls + PSUM evac + ab for half-chunk g
            h3 = h3_tiles[j]
            m0t = m0_tiles[j]
            if g == 0:
                aP = abp.tile([128, POOL_W * NUM_IRREPS], BF16, tag="abP",
                              name=f"abP_{j}", bufs=4)
                aD = abp.tile([128, (8 - POOL_W) * NUM_IRREPS], BF16,
                              tag="abD", name=f"abD_{j}", bufs=4)
                ab_tiles[j] = (aP, aD)
            abt = ab_tiles[j][g]
            nw = POOL_W if g == 0 else 8 - POOL_W
            abv = abt[:].rearrange("p (w l c) -> p w l c", l=3, c=MUL)
            # mix: edge-major via stationary-h3 matmuls, bf16 full rate;
            # 256-col PSUM slots so each output stays inside one bank
            mixt = mixp.tile([128, 4 * 256], F32, tag="mixt")
            for t4 in range(4):
                t = g * 4 + t4
                half, coff = ((0, t * 128) if t < 4
                              else (64, (t - 4) * 128))
                nc.tensor.matmul(
                    mixt[:, t4 * 256:t4 * 256 + NUM_IRREPS],
                    h3[half:half + 64, coff:coff + 128],
                    w3et[half:half + 64, :])
            # evacuate mix PSUM -> bf16 SBUF (ACT)
            mixs = mxs.tile([128, 4 * NUM_IRREPS], BF16, tag="mixs")
            nc.scalar.activation(
                mixs[:].rearrange("p (w x) -> p w x", x=192),
                mixt[:].rearrange("p (w x) -> p w x", x=256)
                [:, :, 0:192], AF.Copy)
            # A = mix (*) msg0 broadcast over l  (all-bf16 SBUF, 2x)
            nc.vector.tensor_tensor(
                abv,
                mixs[:].rearrange("p (w l c) -> p w l c", l=3, c=MUL),
                m0t[:].rearrange("p (w c) -> p w c", c=MUL)
                    [:, g * 4:g * 4 + nw]
                    .unsqueeze(2).broadcast_to([128, nw, 3, MUL]),
                OP.mult)

        def tp_chunk(j):
            # tensor products for chunk j (ab computed last iteration, so
            # both engines launch at iteration start with zero waits).
            # GPSIMD owns windows 0:4, DVE owns 4:8, disjoint msg tiles.
            h3_tiles.pop(j, None)
            q = strip_of_chunk[j]
            wbase = (j - strip_chunks[q][0]) * 8
            y8v = y8_tiles[q][:].rearrange("p (w k) -> p w k", k=8)
            mP = msgp.tile([128, POOL_W * 512], BF16, tag="msgP", bufs=4)
            mD = msgp.tile([128, (8 - POOL_W) * 512], BF16, tag="msgD",
                           bufs=4)
            msg_tiles[j] = (mP, mD)
            aP, aD = ab_tiles[j]
            for g in range(2):
                nw = POOL_W if g == 0 else 8 - POOL_W
                abv = (aP if g == 0 else aD)[:].rearrange(
                    "p (w l c) -> p w l c", l=3, c=MUL)
                eng = nc.gpsimd if g == 0 else nc.vector
                mt = mP if g == 0 else mD
                msgv = mt[:].rearrange("p (w f) -> p w f", f=512)
                y8c = y8v[:, wbase + g * 4:wbase + g * 4 + nw]
                eng.tensor_tensor(
                    msgv[:, :, 0:3 * MUL].rearrange(
                        "p w (c k) -> p w c k", k=3),
                    y8c[:, :, 0:3].unsqueeze(2)
                        .broadcast_to([128, nw, MUL, 3]),
                    abv[:, :, 1, :].unsqueeze(3)
                        .broadcast_to([128, nw, MUL, 3]),
                    OP.mult)
                eng.tensor_tensor(
                    msgv[:, :, 3 * MUL:8 * MUL].rearrange(
                        "p w (c k) -> p w c k", k=5),
                    y8c[:, :, 3:8].unsqueeze(2)
                        .broadcast_to([128, nw, MUL, 5]),
                    abv[:, :, 2, :].unsqueeze(3)
                        .broadcast_to([128, nw, MUL, 5]),
                    OP.mult)

        # Fully software-pipelined main loop. At iteration j (emission
        # order = per-engine queue order):
        #   tp(j-1)            DVE+GPSIMD: deps one iteration old -> both
        #                      engines start instantly at iteration top
        #   h1(j+3)/h2(j+2)/h3(j+1)  PE matmuls with iteration-old inputs
        #   mix/ab(j)          ACT order [silu1, evac-g0, silu2, silu3,
        #                      evac-g1] releases the g1 mix matmuls early;
        #                      segment pairs cover the PSUM turnaround
        #   segment(j-3)       always-ready one-hot matmul block
        strip_starts = {strip_chunks[q][0]: q for q in range(NSTRIP)}
        for j in range(-3, CH + 3):
            if 0 <= j - 1 < CH:
                tp_chunk(j - 1)
            if j + 2 in strip_starts and j + 2 > 0:
                prefetch_vt(strip_starts[j + 2])
            if 0 <= j + 3 < CH:
                prefetch_chunk(j + 3)
                mlp_h1(j + 3)
            if 0 <= j < CH:
                mix_g(j, 0)
            if 0 <= j + 2 < CH:
                mlp_h2(j + 2)
            if 0 <= j + 1 < CH:
                mlp_h3(j + 1)
            seg_part(j - 3, take=3)
            if 0 <= j < CH:
                mix_g(j, 1)
            seg_part(j - 3)
            if j + 1 in strip_starts:
                sph_strip(strip_starts[j + 1])
        # empty blocks (defensive): write zeros
        empty = [b for b in range(BLOCKS) if meta["B_HI"][b] < meta["B_LO"][b]]
        if empty:
            zt = const.tile([128, MSG_W], F32)
            nc.vector.memset(zt[:], 0.0)
            for b in empty:
                nc.sync.dma_start(out_d[b * 128:(b + 1) * 128, :], zt[:])
    nc.compile()
    return nc


def kernel(**inputs) -> np.ndarray:
    in_maps, meta = _prep(**inputs)
    nc = _build(meta)
    from concourse.bass_utils import run_bass_kernel_spmd
    res = run_bass_kernel_spmd(nc, in_maps, list(range(N_CORES)))
    outs = [np.asarray(res.results[c]["out"], np.float32)
            for c in range(N_CORES)]
    return np.concatenate(outs, axis=0)


if __name__ == "__main__":
    import reference
    ins = {k: np.asarray(v) for k, v in reference.setup_inputs().items()}
    out = kernel(**ins)
    exp = np.asarray(reference.reference(**reference.setup_inputs()))
    err = np.abs(out - exp).max() / np.abs(exp).max()
    print("rel err:", err)
